# revision 8
# baseline (speedup 1.0000x reference)
"""Multi-head attention (softmax over query axis) on 8 Trainium2 cores.

Problem: nn_MultiHeadAttention_3899830305178
  B=2, S=2048, D_MODEL=1024, HEADS=16, D_K=64, fp32 IO.
  reference:
    q = (query @ Wq + bq), k = ..., v = ...        [b, s, h, dk]
    scores = einsum('bihd,bjhd->bijh', q, k) / 8
    attn = softmax(scores, axis=1)                 # over QUERY axis i (quirk)
    x = einsum('bijh,bjhd->bihd', attn, v)         [b, s, h*dk]
    out = x @ Wo + bo

Sharding: data-parallel over batch (2) x tensor-parallel over heads (4 groups
of 4 heads) = 8 cores. Each core computes a partial output
O_part = x_local @ Wo[rows of its heads]; the host sums the 4 partials per
batch (row-parallel unshard) -- bo is added on-device by the g==0 core.

Per-core kernel math (host passes query/key/value pre-transposed so the
projections contract over the model dim on partitions):
  qT[d', i] = Wq_s.T @ queryT      (d' = 4 local heads x 64 = 256)
  kT[d', j] = Wk_s.T @ keyT
  vT[d', j] = Wv_s.T @ valueT (+bv), then bf16 DMA-transpose -> v[j, d']
  per head h:  sT[j, i] = kT_h.T @ qT_h / 8  (softmax over i == free axis)
               eT = exp(sT) (bf16), rowsum via a DVE tensor_scalar accum
               v_h_scaled[j, :] = v_h[j, :] / rowsum[j]   <- softmax divisor
               xT_h[d, i] = v_h_scaled.T @ eT             (contracts over j)
  O_part[i, n] = xT.T @ Wo_s; bo is added on the host during unshard.

Engine balance (per CoreSim, 230us single-shot span): PE ~169us
(projections 41 + scores 55 + attn@V 55 + out-proj 14), ACT ~138us (the
128 [128,1024] exps are irreducible -- Exp exists only on ACT), DVE
~93us, 16KB/partition PSUM exactly full (2x scores buffers + 2x
proj/attn@V buffers). Startup: wq/wk load on the (initially idle) ACT
HWDGE queue in parallel with the q/k input chunks on SP; the q/k
projection stream is ordered q0,q1,k0,q2,q3,k1..k3 to match the ACT
engine's exp demand order; wv/wo load late; qT is tiled per input chunk
so each scores matmul gates on exactly one projected chunk.
Design choices vs the naive version:
  - sibling heads (partitions 0-63 / 64-127 of the kT/qT slices) emit
    their K=64 scores matmuls interleaved with PE tile positions
    (0,0)/(64,0), letting the PE overlap row tiles (K=64 alone half-fills
    the 128x128 array);
  - eT is bf16 (same PE rate as f32r, half the SBUF/attp footprint);
  - softmax rowsums come from a DVE tensor_scalar (2-byte fast mode,
    ~0.4us/tile) instead of the ACT accum_out (+187ns/tile on the other
    near-critical engine), computed as out=(a*1)+0 in place with
    accum_out=rowsum;
  - attn@V is split by i-half into [64,1024] psums so the first half's
    output projection overlaps the second half's attention (xT is split
    per i-half so the dependency is tile-precise); pair 1's first scores
    are emitted inside pair 0's phase B so the ACT engine never idles
    there, and the first-half output projection (with O DMAs alternating
    between the SP and ACT hardware queues) drains during pair 1's
    phase B;
  - the bias matmuls (K=1 ones-row) were removed from the PE: bo rides
    the host-side unshard sum.

Projection inputs/weights are bf16; scores and the output projection run
in float32r (TF32, fp32 accumulate) with fp32 softmax statistics; attn@V
is bf16 x bf16 -> fp32. Measured end-to-end relative error vs the fp64
reference is ~4.4e-3 on hardware.
"""

import numpy as np

import concourse.bass as bass
import concourse.mybir as mybir
import concourse.tile as tile
from concourse.bass_utils import run_bass_kernel_spmd

# problem shape (hardcoded per contract)
B, S, DM, H, DK = 2, 2048, 1024, 16, 64
N_CORES = 8
GROUPS = 4              # head groups (tensor-parallel)
HL = H // GROUPS        # 4 local heads per core
DL = HL * DK            # 256 local concat width
P = 128
SJ = S // P             # 16 strips of 128 along j (keys) and i (out rows)
MT = DM // P            # 8 contraction tiles for projections
DPT = DL // P           # 2 partition tiles of the local concat dim
SCALE = 1.0 / 8.0       # 1/sqrt(DK)

f32 = mybir.dt.float32
f32r = mybir.dt.float32r
bf16 = mybir.dt.bfloat16
AF = mybir.ActivationFunctionType

# Projection stage (inputs + projection weights) in bf16: halves the input
# DMA (the critical-path prefix) at ~2e-3 relative error. Attention and
# output projection stay TF32.
PROJ_BF16 = True
PROJ_DT = bf16 if PROJ_BF16 else f32r

# Interleave sibling-head scores matmuls (ABAB) so their PE row tiles
# (0,0)/(64,0) are adjacent and can overlap; 0 emits AABB for comparison.
import os as _os
PAIR_INTERLEAVE = _os.environ.get("PAIR_INTERLEAVE", "1") == "1"

_PROGRAM = None


def _split_excess_waits(nc, max_waits=1):
    """walrus in this container rejects >1 semaphore wait per instruction
    (e.g. the Tile kernel-tail Drain); move extras onto same-engine NOPs."""
    n_split = 0
    for f in nc.m.functions:
        for blk in f.blocks:
            new_insts = []
            for inst in blk.instructions:
                si = getattr(inst, "sync_info", None)
                if si is not None and si.on_wait and len(si.on_wait) > max_waits:
                    waits = list(si.on_wait)
                    extra, keep = waits[:-max_waits], waits[-max_waits:]
                    for i in range(0, len(extra), max_waits):
                        chunk = extra[i:i + max_waits]
                        nop = mybir.InstNoOp(
                            name=f"{inst.name}-ws{n_split}-{i}",
                            engine=inst.engine,
                            sync_info=mybir.SyncInfo(on_wait=chunk, on_update=[]),
                            bass_nofuse=True,
                        )
                        new_insts.append(nop)
                    si.on_wait = keep
                    n_split += 1
                new_insts.append(inst)
            blk.instructions[:] = new_insts
    return n_split


def emit_iter(nc, tc, it, const, sb, stat, outp, inp, vtp, attp, pps, ppx,
              qT_in, kT_in, vT_in, wo_d, O_d,
              wq_sb, wk_sb, load_wv, bq_sb, bk_sb, bv_sb, w_slice, wo_cell):
    """One full attention iteration (tile names suffixed _r{it} so the
    program body can be repeated for steady-state timing; tags are shared
    so pool buffers rotate/serialize across reps)."""
    R = f"_r{it}"

    # ---------------- persistent activations ----------------
    # qT split by i-half, kT by i-quarter (j-group): finer tiles give
    # the scheduler finer dependencies, so scores start before the
    # whole projection finishes.
    qT_sb = [[sb.tile([P, 512], f32r, name=f"qT{dp}_{i4}{R}",
                      tag=f"qT{dp}_{i4}") for i4 in range(4)]
             for dp in range(DPT)]
    kT_sb = [[sb.tile([P, 512], f32r, name=f"kT{dp}_{jg}{R}",
                      tag=f"kT{dp}_{jg}") for jg in range(4)]
             for dp in range(DPT)]
    # v packed per j-group of 4: v4_sb[jg][p, jj*DL + d'] holds
    # v[jg*512 + jj*128 + p, d']
    v4_sb = [sb.tile([P, 4 * DL], bf16, name=f"v{jg}{R}", tag=f"v{jg}")
             for jg in range(4)]
    xT_sb = [[sb.tile([P, 1024], f32r, name=f"xT{hp}_{ih}{R}",
                      tag=f"xT{hp}_{ih}") for ih in range(2)]
             for hp in range(DPT)]

    # ---------------- projections ----------------
    # dst[d', i] = W.T @ inT ; contraction over m on partitions.
    vT_sb = [vtp.tile([P, S], bf16, name=f"vT{dp}{R}", tag=f"vT{dp}")
             for dp in range(DPT)]

    ENGQ = {"q": nc.sync, "k": nc.sync, "v": nc.sync}

    def load_in_chunk(win, nm, i4):
        # one DMA: all 8 m-blocks of columns [i0, i0+512)
        t = inp.tile([P, MT * 512], PROJ_DT, name=f"{nm}in{i4}{R}",
                     tag="pin")
        src = win.ap().rearrange("(t p) c -> p t c", p=P)
        ENGQ[nm].dma_start(
            t[:].rearrange("p (t c) -> p t c", t=MT),
            src[:, :, i4 * 512:(i4 + 1) * 512])
        return t

    qk_prio = tc.high_priority()
    qk_prio.__enter__()
    QK_ORDER = [("q", 0), ("k", 0), ("q", 1), ("q", 2), ("q", 3),
                ("k", 1), ("k", 2), ("k", 3)]
    for nm, i4 in QK_ORDER:
        if True:
            win, w_sb, b_sb = ((qT_in, wq_sb, bq_sb) if nm == "q"
                               else (kT_in, wk_sb, bk_sb))
            ch = load_in_chunk(win, nm, i4)
            ps = ppx.tile([P, 1024], f32, name=f"ps{nm}{i4}{R}",
                          tag="px", bufs=2)
            for dp in range(DPT):
                for m in range(MT):
                    nc.tensor.matmul(
                        ps[:, dp * 512:(dp + 1) * 512],
                        w_slice(w_sb, m, dp),
                        ch[:, m * 512:(m + 1) * 512],
                        start=(m == 0), stop=(m == MT - 1))
            for dp in range(DPT):
                if nm == "q":
                    dst = qT_sb[dp][i4][:]
                else:
                    dst = kT_sb[dp][i4][:]
                nc.vector.tensor_scalar_add(
                    dst, ps[:, dp * 512:(dp + 1) * 512],
                    b_sb[:, dp:dp + 1])

    # vT[d', j] = Wv.T @ valueT (bias folded in, bf16 out), then one
    # SBUF->SBUF bf16 DMA-transpose per (i4, dp) covering 4 j-tiles,
    # dispatched on the ACT HWDGE queue to keep SP free for inputs.
    qk_prio.__exit__(None, None, None)
    wv_sb = load_wv()
    for i4 in range(4):
        i0 = i4 * 512
        ch = load_in_chunk(vT_in, "v", i4)
        ps = ppx.tile([P, 1024], f32, name=f"psvt{i4}{R}", tag="px", bufs=2)
        for dp in range(DPT):
            for m in range(MT):
                nc.tensor.matmul(
                    ps[:, dp * 512:(dp + 1) * 512],
                    w_slice(wv_sb, m, dp),
                    ch[:, m * 512:(m + 1) * 512],
                    start=(m == 0), stop=(m == MT - 1))
        for dp in range(DPT):
            nc.vector.tensor_scalar_add(
                vT_sb[dp][:, i0:i0 + 512],
                ps[:, dp * 512:(dp + 1) * 512], bv_sb[:, dp:dp + 1])
        for dp in range(DPT):
            out_view = v4_sb[i4][:].rearrange(
                "p (j c) -> p j c", j=4)[:, :,
                                         dp * P:(dp + 1) * P]
            # sync queue: keeps the ACT queue free -- ACT (exp) is the
            # whole-kernel critical path and each queued DMA costs ~1.2us
            # of its engine time.
            nc.sync.dma_start(
                out_view, vT_sb[dp][:, i0:i0 + 512], transpose=True)

    # ---------------- attention (per head pair) ----------------
    # Heads run in sibling pairs (2hp, 2hp+1) whose kT/qT slices live at
    # partitions 0-63 / 64-127: their K=64 scores matmuls carry PE tile
    # positions (0,0)/(64,0) and are emitted interleaved so the PE can
    # overlap the row tiles (K=64 alone half-fills the array).
    # exp writes bf16 eT tiles WITHOUT accum_out; the softmax row sums come
    # from a DVE in-place tensor_scalar (4x bf16 mode) so the ACT engine --
    # the other near-critical engine -- sheds the 187ns/tile accumulator
    # reads. attn@V is split by i-half: each half accumulates into its own
    # [64, 1024] psum (2 tiles = the pair), which keeps PSUM at 16KB and
    # lets the first i-half's output projection overlap the second half.

    MULT = mybir.AluOpType.mult
    ADD = mybir.AluOpType.add

    def pair_scores(hp, j, ih):
        jg, jr = divmod(j, 4)
        psA = pps.tile([P, 1024], f32, name=f"psa{hp}_{j}_{ih}{R}",
                       tag="ps")
        psB = pps.tile([P, 1024], f32, name=f"psb{hp}_{j}_{ih}{R}",
                       tag="ps")
        if PAIR_INTERLEAVE:
            order = [(i5, hh) for i5 in range(2) for hh in range(2)]
        else:
            order = [(i5, hh) for hh in range(2) for i5 in range(2)]
        for i5, hh in order:
            io = i5 * 512
            ps = psA if hh == 0 else psB
            base = hh * 64
            nc.tensor.matmul(
                ps[:, io:io + 512],
                kT_sb[hp][jg][base:base + 64, jr * P:(jr + 1) * P],
                qT_sb[hp][ih * 2 + i5][base:base + 64, :],
                start=True, stop=True)
        outs = []
        for hh, ps in ((0, psA), (1, psB)):
            h = hp * 2 + hh
            a = attp.tile([P, 1024], bf16, name=f"att{h}_{j}_{ih}{R}",
                          tag=f"att{ih}", bufs=(11 if ih == 0 else 33))
            rsh = stat.tile([P, 1], f32, name=f"rsh{h}_{j}_{ih}{R}",
                            tag="rsh", bufs=48)
            # exp + softmax rowsum in ONE ACT instruction: the HW DVE runs
            # the tensor_scalar cache-reduce at 1x (1.2us/tile measured vs
            # 0.4us modeled), which made DVE the whole-kernel bottleneck;
            # the ACT accumulator costs only ~190ns/tile on this stream.
            nc.scalar.activation(a[:], ps[:], AF.Exp, scale=SCALE,
                                 accum_out=rsh[:])
            outs.append((a, rsh))
        return outs

    def finish_head(hp, hh, j, xps, ih, ah):
        # attn@V for one head and one i-half: 2 matmuls of 512 rows.
        # Sibling heads write partitions 0-63 / 64-127 of a shared psum
        # (tile_position col 0/64), so their M=64 matmuls overlap on the PE.
        for i5 in range(2):
            io = i5 * 512
            nc.tensor.matmul(
                xps[hh * 64:(hh + 1) * 64, io:io + 512],
                vsc_t[hp * 2 + hh][j][:],
                ah[:, io:io + 512],
                start=(j == 0), stop=(j == SJ - 1),
                skip_group_check=True)

    def make_vsc(hp, hh, j, rs_halves):
        h = hp * 2 + hh
        jg, jr = divmod(j, 4)
        rs = stat.tile([P, 1], f32, name=f"rs{h}_{j}{R}", tag="rs")
        nc.vector.tensor_add(rs[:], rs_halves[0][:], rs_halves[1][:])
        rc = stat.tile([P, 1], f32, name=f"rc{h}_{j}{R}", tag="rc")
        nc.vector.reciprocal(rc[:], rs[:])
        vsc = attp.tile([P, 64], bf16, name=f"vsc{h}_{j}{R}", tag="vsc",
                        bufs=34)
        nc.vector.tensor_scalar_mul(
            vsc[:],
            v4_sb[jg][:, jr * DL + h * 64:jr * DL + (h + 1) * 64],
            rc[:])
        vsc_t[h][j] = vsc

    vsc_t = [[None] * SJ for _ in range(HL)]
    a1_t = [[None] * SJ for _ in range(HL)]

    # ---------------- output projection ----------------
    # bo is added on the host during unshard (a K=1 ones-row matmul for it
    # here would cost 16384 PE rows ~ 7us).
    # wo is first needed here -- loading it now keeps the front DMA
    # bandwidth for the projection inputs.
    if not wo_cell:
        wo_sb = const.tile([P, DPT * DM], f32r, name="wo", tag="wo")
        nc.sync.dma_start(
            wo_sb[:].rearrange("p (t c) -> p t c", t=DPT),
            wo_d.ap().rearrange("(t p) c -> p t c", p=P))
        wo_cell.append(wo_sb)
    wo_sb = wo_cell[0]

    def emit_outproj(jts):
        for jt in jts:
            ot = outp.tile([P, DM], f32, name=f"ot{jt}{R}", tag="ot")
            for n5 in range(2):
                no = n5 * 512
                ps = pps.tile([P, 512], f32, name=f"pso{jt}_{n5}{R}",
                              tag="ps")
                jh, jo = divmod(jt, 8)
                for cpt in range(DPT):
                    nc.tensor.matmul(
                        ps[:], xT_sb[cpt][jh][:, jo * P:(jo + 1) * P],
                        wo_sb[:, cpt * DM + no:cpt * DM + no + 512],
                        start=(cpt == 0), stop=(cpt == DPT - 1))
                nc.vector.tensor_copy(ot[:, no:no + 512], ps[:])
            oq = nc.sync if jt % 2 == 0 else nc.gpsimd
            oq.dma_start(O_d.ap()[jt * P:(jt + 1) * P, :], ot[:])


    PRE = 4  # pair-1 scores emitted inside pair-0's phase B (ACT filler)

    def phase_a_j(hp, j, xph0, o0, o1):
        for hh in range(2):
            a1_t[hp * 2 + hh][j] = o1[hh][0]
            make_vsc(hp, hh, j, [o0[hh][1], o1[hh][1]])
            finish_head(hp, hh, j, xph0, 0, o0[hh][0])

    def alloc_xps(hp, ih):
        return ppx.tile([P, 1024], f32, name=f"xp{ih}_{hp}{R}", tag="px",
                        bufs=2)

    def copy_xps(hp, ih, xph):
        nc.vector.tensor_copy(xT_sb[hp][ih][:], xph[:])

    prio = tc.high_priority()
    prio.__enter__()

    # ---- pair 0, phase A: overlaps the projections; emit ih=0 exps ahead
    # of ih=1 (which needs the later-projected qT half) in j-quarters.
    xph0 = alloc_xps(0, 0)
    saved = {}
    for jb in range(4):
        js = range(jb * 4, jb * 4 + 4)
        for j in js:
            saved[j] = pair_scores(0, j, 0)
        for j in js:
            o1 = pair_scores(0, j, 1)
            phase_a_j(0, j, xph0, saved[j], o1)
    copy_xps(0, 0, xph0)

    # ---- pair 0 phase B, interleaved with pair 1's first PRE j's of
    # scores+exp: phase B is pure PE work, and without the filler the ACT
    # engine (the attention pacer) would idle for its whole span.
    xph1 = alloc_xps(0, 1)
    saved01 = {}
    for j in range(SJ):
        for hh in range(2):
            finish_head(0, hh, j, xph1, 1, a1_t[hh][j])
        if j % 4 == 0 and j // 4 < PRE:
            jp = j // 4
            saved01[jp] = (pair_scores(1, jp, 0), pair_scores(1, jp, 1))
    copy_xps(0, 1, xph1)

    # ---- pair 1 phase A: deferred finishes for the PRE j's, then the rest
    xph0 = alloc_xps(1, 0)
    for j in range(PRE):
        phase_a_j(1, j, xph0, saved01[j][0], saved01[j][1])
    for j in range(PRE, SJ):
        o0 = pair_scores(1, j, 0)
        o1 = pair_scores(1, j, 1)
        phase_a_j(1, j, xph0, o0, o1)
    copy_xps(1, 0, xph0)

    # ---- output projection for the first i-half (needs only xT[.][0]),
    # emitted before pair 1's phase B so its O DMAs drain during that
    # pure-PE stretch
    emit_outproj(range(SJ // 2))

    # ---- pair 1 phase B
    xph1 = alloc_xps(1, 1)
    for j in range(SJ):
        for hh in range(2):
            finish_head(1, hh, j, xph1, 1, a1_t[2 + hh][j])
    copy_xps(1, 1, xph1)
    prio.__exit__(None, None, None)

    emit_outproj(range(SJ // 2, SJ))



def build_program(split_waits=True, reps=1):
    nc = bass.Bass("TRN2", target_bir_lowering=False, debug=False)

    qT_in = nc.dram_tensor("qT_in", [DM, S], PROJ_DT, kind="ExternalInput")
    kT_in = nc.dram_tensor("kT_in", [DM, S], PROJ_DT, kind="ExternalInput")
    vT_in = nc.dram_tensor("vT_in", [DM, S], PROJ_DT, kind="ExternalInput")
    wq_d = nc.dram_tensor("wq", [DM, DL], PROJ_DT, kind="ExternalInput")
    wk_d = nc.dram_tensor("wk", [DM, DL], PROJ_DT, kind="ExternalInput")
    wv_d = nc.dram_tensor("wv", [DM, DL], PROJ_DT, kind="ExternalInput")
    wo_d = nc.dram_tensor("wo", [DL, DM], f32r, kind="ExternalInput")
    bq_d = nc.dram_tensor("bq", [DL, 1], f32, kind="ExternalInput")
    bk_d = nc.dram_tensor("bk", [DL, 1], f32, kind="ExternalInput")
    bv_d = nc.dram_tensor("bv", [DL, 1], f32, kind="ExternalInput")
    O_d = nc.dram_tensor("O", [S, DM], f32, kind="ExternalOutput")

    with tile.TileContext(nc) as tc:
        with (
            tc.tile_pool(name="const", bufs=1) as const,
            tc.tile_pool(name="persist", bufs=1) as sb,
            tc.tile_pool(name="stat", bufs=6) as stat,
            tc.tile_pool(name="outp", bufs=3) as outp,
            tc.tile_pool(name="inp", bufs=2) as inp,
            tc.tile_pool(name="vtp", bufs=1) as vtp,
            tc.tile_pool(name="attp", bufs=20) as attp,
            tc.tile_pool(name="pps", bufs=2, space="PSUM") as pps,
            tc.tile_pool(name="ppx", bufs=1, space="PSUM") as ppx,
        ):
            # ---------------- constants ----------------
            # One DMA per weight: DRAM [(t p), c] -> SBUF [p, (t c)] so the
            # m-th 128-row block lands at free offset m*DL.
            def load_w(dram, nm, dt_, cols, eng):
                t = const.tile([P, MT * cols], dt_, name=nm, tag=nm)
                eng.dma_start(
                    t[:].rearrange("p (t c) -> p t c", t=MT),
                    dram.ap().rearrange("(t p) c -> p t c", p=P))
                return t

            wq_sb = load_w(wq_d, "wq", PROJ_DT, DL, nc.scalar)  # [128, 8*256]
            wk_sb = load_w(wk_d, "wk", PROJ_DT, DL, nc.scalar)
            bq_sb = const.tile([P, DPT], f32, name="bq", tag="bq")
            nc.sync.dma_start(
                bq_sb[:].rearrange("p (t c) -> p t c", t=DPT),
                bq_d.ap().rearrange("(t p) c -> p t c", p=P))
            bk_sb = const.tile([P, DPT], f32, name="bk", tag="bk")
            nc.sync.dma_start(
                bk_sb[:].rearrange("p (t c) -> p t c", t=DPT),
                bk_d.ap().rearrange("(t p) c -> p t c", p=P))
            bv_sb = const.tile([P, DPT], f32, name="bv", tag="bv")
            nc.sync.dma_start(
                bv_sb[:].rearrange("p (t c) -> p t c", t=DPT),
                bv_d.ap().rearrange("(t p) c -> p t c", p=P))
            wv_cell = []

            def load_wv():
                if not wv_cell:
                    wv_cell.append(load_w(wv_d, "wv", PROJ_DT, DL, nc.sync))
                return wv_cell[0]

            def w_slice(w, m, dp):
                return w[:, m * DL + dp * P:m * DL + (dp + 1) * P]

            wo_cell = []

            for it in range(reps):
                emit_iter(nc, tc, it, const, sb, stat, outp, inp, vtp, attp,
                          pps, ppx, qT_in, kT_in, vT_in, wo_d, O_d,
                          wq_sb, wk_sb, load_wv, bq_sb, bk_sb, bv_sb, w_slice,
                          wo_cell)

    if split_waits:
        _split_excess_waits(nc)
    return nc


def _get_program():
    global _PROGRAM
    if _PROGRAM is None:
        _PROGRAM = build_program()
    return _PROGRAM


def _tf32(x):
    """Round fp32 -> TF32 (10-bit mantissa), round-to-nearest-even."""
    x = np.ascontiguousarray(np.asarray(x, dtype=np.float32))
    u = x.view(np.uint32)
    r = ((u >> 13) & 1).astype(np.uint32)
    u2 = ((u + np.uint32(0x0FFF) + r) & np.uint32(0xFFFFE000))
    return u2.view(np.float32)


def shard_inputs(inputs):
    """FULL inputs -> per-core in_maps (list of 8 dicts)."""
    q = np.asarray(inputs["query"], dtype=np.float32)
    k = np.asarray(inputs["key"], dtype=np.float32)
    v = np.asarray(inputs["value"], dtype=np.float32)
    Wq = np.asarray(inputs["Wq"], dtype=np.float32)
    Wk = np.asarray(inputs["Wk"], dtype=np.float32)
    Wv = np.asarray(inputs["Wv"], dtype=np.float32)
    Wo = np.asarray(inputs["Wo"], dtype=np.float32)
    bq = np.asarray(inputs["bq"], dtype=np.float32)
    bk = np.asarray(inputs["bk"], dtype=np.float32)
    bv = np.asarray(inputs["bv"], dtype=np.float32)
    bo = np.asarray(inputs["bo"], dtype=np.float32)

    if PROJ_BF16:
        import ml_dtypes

        def _proj_cast(x):
            return np.ascontiguousarray(np.asarray(x, np.float32)).astype(
                ml_dtypes.bfloat16)
    else:
        _proj_cast = _tf32

    qT = [_proj_cast(q[b].T) for b in range(B)]
    kT = [_proj_cast(k[b].T) for b in range(B)]
    vT = [_proj_cast(v[b].T) for b in range(B)]

    in_maps = []
    for c in range(N_CORES):
        b, g = c // GROUPS, c % GROUPS
        sl = slice(g * DL, (g + 1) * DL)
        in_maps.append({
            "qT_in": qT[b],
            "kT_in": kT[b],
            "vT_in": vT[b],
            "wq": _proj_cast(Wq[:, sl]),
            "wk": _proj_cast(Wk[:, sl]),
            "wv": _proj_cast(Wv[:, sl]),
            "wo": _tf32(Wo[sl, :]),
            "bq": np.ascontiguousarray(bq[sl].reshape(DL, 1)),
            "bk": np.ascontiguousarray(bk[sl].reshape(DL, 1)),
            "bv": np.ascontiguousarray(bv[sl].reshape(DL, 1)),
        })
    return in_maps


def unshard_output(results, bo):
    """results: list of 8 dicts with 'O' [S, DM] -> full [B, S, DM].
    bo is added here (host) -- cheaper than a K=1 PE matmul on device."""
    out = np.zeros((B, S, DM), np.float32)
    for c in range(N_CORES):
        out[c // GROUPS] += results[c]["O"]
    out += np.asarray(bo, np.float32)
    return out


def kernel(**inputs):
    nc = _get_program()
    in_maps = shard_inputs(inputs)
    res = run_bass_kernel_spmd(nc, in_maps, core_ids=list(range(N_CORES)))
    return unshard_output(res.results, inputs["bo"])



# revision 12
# speedup vs baseline: 1.1105x; 1.1105x over previous
"""Multi-head attention (softmax over query axis) on 8 Trainium2 cores.

Problem: nn_MultiHeadAttention_3899830305178
  B=2, S=2048, D_MODEL=1024, HEADS=16, D_K=64, fp32 IO.
  reference:
    q = (query @ Wq + bq), k = ..., v = ...        [b, s, h, dk]
    scores = einsum('bihd,bjhd->bijh', q, k) / 8
    attn = softmax(scores, axis=1)                 # over QUERY axis i (quirk)
    x = einsum('bijh,bjhd->bihd', attn, v)         [b, s, h*dk]
    out = x @ Wo + bo

Sharding: data-parallel over batch (2) x tensor-parallel over heads (4 groups
of 4 heads) = 8 cores. Each core computes a partial output
O_part = x_local @ Wo[rows of its heads]; the host sums the 4 partials per
batch (row-parallel unshard) -- bo is added on-device by the g==0 core.

Per-core kernel math (host passes query/key/value pre-transposed so the
projections contract over the model dim on partitions):
  qT[d', i] = Wq_s.T @ queryT      (d' = 4 local heads x 64 = 256)
  kT[d', j] = Wk_s.T @ keyT
  vT[d', j] = Wv_s.T @ valueT (+bv), then bf16 DMA-transpose -> v[j, d']
  per head h:  sT[j, i] = kT_h.T @ qT_h / 8  (softmax over i == free axis)
               eT = exp(sT) (bf16), rowsum via a DVE tensor_scalar accum
               v_h_scaled[j, :] = v_h[j, :] / rowsum[j]   <- softmax divisor
               xT_h[d, i] = v_h_scaled.T @ eT             (contracts over j)
  O_part[i, n] = xT.T @ Wo_s; bo is added on the host during unshard.

Engine balance (per CoreSim, 230us single-shot span): PE ~169us
(projections 41 + scores 55 + attn@V 55 + out-proj 14), ACT ~138us (the
128 [128,1024] exps are irreducible -- Exp exists only on ACT), DVE
~93us, 16KB/partition PSUM exactly full (2x scores buffers + 2x
proj/attn@V buffers). Startup: wq/wk load on the (initially idle) ACT
HWDGE queue in parallel with the q/k input chunks on SP; the q/k
projection stream is ordered q0,q1,k0,q2,q3,k1..k3 to match the ACT
engine's exp demand order; wv/wo load late; qT is tiled per input chunk
so each scores matmul gates on exactly one projected chunk.
Design choices vs the naive version:
  - sibling heads (partitions 0-63 / 64-127 of the kT/qT slices) emit
    their K=64 scores matmuls interleaved with PE tile positions
    (0,0)/(64,0), letting the PE overlap row tiles (K=64 alone half-fills
    the 128x128 array);
  - eT is bf16 (same PE rate as f32r, half the SBUF/attp footprint);
  - softmax rowsums come from a DVE tensor_scalar (2-byte fast mode,
    ~0.4us/tile) instead of the ACT accum_out (+187ns/tile on the other
    near-critical engine), computed as out=(a*1)+0 in place with
    accum_out=rowsum;
  - attn@V is split by i-half into [64,1024] psums so the first half's
    output projection overlaps the second half's attention (xT is split
    per i-half so the dependency is tile-precise); pair 1's first scores
    are emitted inside pair 0's phase B so the ACT engine never idles
    there, and the first-half output projection (with O DMAs alternating
    between the SP and ACT hardware queues) drains during pair 1's
    phase B;
  - the bias matmuls (K=1 ones-row) were removed from the PE: bo rides
    the host-side unshard sum.

Projection inputs/weights are bf16; scores and the output projection run
in float32r (TF32, fp32 accumulate) with fp32 softmax statistics; attn@V
is bf16 x bf16 -> fp32. Measured end-to-end relative error vs the fp64
reference is ~4.4e-3 on hardware.
"""

import numpy as np

import concourse.bass as bass
import concourse.mybir as mybir
import concourse.tile as tile
from concourse.bass_utils import run_bass_kernel_spmd

# problem shape (hardcoded per contract)
B, S, DM, H, DK = 2, 2048, 1024, 16, 64
N_CORES = 8
GROUPS = 4              # head groups (tensor-parallel)
HL = H // GROUPS        # 4 local heads per core
DL = HL * DK            # 256 local concat width
P = 128
SJ = S // P             # 16 strips of 128 along j (keys) and i (out rows)
MT = DM // P            # 8 contraction tiles for projections
DPT = DL // P           # 2 partition tiles of the local concat dim
SCALE = 1.0 / 8.0       # 1/sqrt(DK)

f32 = mybir.dt.float32
f32r = mybir.dt.float32r
bf16 = mybir.dt.bfloat16
AF = mybir.ActivationFunctionType

# Projection stage (inputs + projection weights) in bf16: halves the input
# DMA (the critical-path prefix) at ~2e-3 relative error. Attention and
# output projection stay TF32.
PROJ_BF16 = True
PROJ_DT = bf16 if PROJ_BF16 else f32r

import os as _os
# Reuse PE stationary weights across same-lhsT matmul runs by suppressing
# the per-matmul LDWEIGHTS (InstMatmult.ldweights=False on the trailing
# matmuls of each run).
LDW_REUSE = _os.environ.get("LDW_REUSE", "1") == "1"
# Which j parity routes its softmax rowsums to the DVE (the other parity
# uses the exp's ACT-side accumulator). 2 = all rowsums on ACT.
ROWSUM_DVE_PARITY = int(_os.environ.get("ROWSUM_DVE_PARITY", "1"))

_PROGRAM = None


def _split_excess_waits(nc, max_waits=1):
    """walrus in this container rejects >1 semaphore wait per instruction
    (e.g. the Tile kernel-tail Drain); move extras onto same-engine NOPs."""
    n_split = 0
    for f in nc.m.functions:
        for blk in f.blocks:
            new_insts = []
            for inst in blk.instructions:
                si = getattr(inst, "sync_info", None)
                if si is not None and si.on_wait and len(si.on_wait) > max_waits:
                    waits = list(si.on_wait)
                    extra, keep = waits[:-max_waits], waits[-max_waits:]
                    for i in range(0, len(extra), max_waits):
                        chunk = extra[i:i + max_waits]
                        nop = mybir.InstNoOp(
                            name=f"{inst.name}-ws{n_split}-{i}",
                            engine=inst.engine,
                            sync_info=mybir.SyncInfo(on_wait=chunk, on_update=[]),
                            bass_nofuse=True,
                        )
                        new_insts.append(nop)
                    si.on_wait = keep
                    n_split += 1
                new_insts.append(inst)
            blk.instructions[:] = new_insts
    return n_split


def emit_iter(nc, tc, it, const, sb, stat, outp, inp, vtp, attp, pps, ppx,
              qT_in, kT_in, vT_in, wo_d, O_d,
              wq_sb, wk_sb, load_wv, bq_sb, bk_sb, bv_sb, w_slice, wo_cell):
    """One full attention iteration (tile names suffixed _r{it} so the
    program body can be repeated for steady-state timing; tags are shared
    so pool buffers rotate/serialize across reps)."""
    R = f"_r{it}"

    # ---------------- persistent activations ----------------
    # qT split by i-half, kT by i-quarter (j-group): finer tiles give
    # the scheduler finer dependencies, so scores start before the
    # whole projection finishes.
    qT_sb = [[sb.tile([P, 512], bf16, name=f"qT{dp}_{i4}{R}",
                      tag=f"qT{dp}_{i4}") for i4 in range(4)]
             for dp in range(DPT)]
    kT_sb = [[sb.tile([P, 512], bf16, name=f"kT{dp}_{jg}{R}",
                      tag=f"kT{dp}_{jg}") for jg in range(4)]
             for dp in range(DPT)]
    # v packed per j-group of 4: v4_sb[jg][p, jj*DL + d'] holds
    # v[jg*512 + jj*128 + p, d']
    v4_sb = [sb.tile([P, 4 * DL], bf16, name=f"v{jg}{R}", tag=f"v{jg}")
             for jg in range(4)]
    xT_sb = [[sb.tile([P, 1024], bf16, name=f"xT{hp}_{ih}{R}",
                      tag=f"xT{hp}_{ih}") for ih in range(2)]
             for hp in range(DPT)]

    # ---------------- projections ----------------
    # dst[d', i] = W.T @ inT ; contraction over m on partitions.
    vT_sb = [vtp.tile([P, S], bf16, name=f"vT{dp}{R}", tag=f"vT{dp}")
             for dp in range(DPT)]

    ENGQ = {"q": nc.sync, "k": nc.sync, "v": nc.sync}

    def load_in_chunk(win, nm, i4):
        # one DMA: all 8 m-blocks of columns [i0, i0+512)
        t = inp.tile([P, MT * 512], PROJ_DT, name=f"{nm}in{i4}{R}",
                     tag="pin")
        src = win.ap().rearrange("(t p) c -> p t c", p=P)
        ENGQ[nm].dma_start(
            t[:].rearrange("p (t c) -> p t c", t=MT),
            src[:, :, i4 * 512:(i4 + 1) * 512])
        return t

    qk_prio = tc.high_priority()
    qk_prio.__enter__()
    QK_ORDER = [("q", 0), ("k", 0), ("q", 1), ("q", 2), ("q", 3),
                ("k", 1), ("k", 2), ("k", 3)]
    for nm, i4 in QK_ORDER:
        if True:
            win, w_sb, b_sb = ((qT_in, wq_sb, bq_sb) if nm == "q"
                               else (kT_in, wk_sb, bk_sb))
            ch = load_in_chunk(win, nm, i4)
            ps = ppx.tile([P, 1024], f32, name=f"ps{nm}{i4}{R}",
                          tag="px", bufs=2)
            for dp in range(DPT):
                for m in range(MT):
                    nc.tensor.matmul(
                        ps[:, dp * 512:(dp + 1) * 512],
                        w_slice(w_sb, m, dp),
                        ch[:, m * 512:(m + 1) * 512],
                        start=(m == 0), stop=(m == MT - 1))
            for dp in range(DPT):
                if nm == "q":
                    dst = qT_sb[dp][i4][:]
                else:
                    dst = kT_sb[dp][i4][:]
                nc.vector.tensor_scalar_add(
                    dst, ps[:, dp * 512:(dp + 1) * 512],
                    b_sb[:, dp:dp + 1])

    # vT[d', j] = Wv.T @ valueT (bias folded in, bf16 out), then one
    # SBUF->SBUF bf16 DMA-transpose per (i4, dp) covering 4 j-tiles,
    # dispatched on the ACT HWDGE queue to keep SP free for inputs.
    qk_prio.__exit__(None, None, None)
    wv_sb = load_wv()
    for i4 in range(4):
        i0 = i4 * 512
        ch = load_in_chunk(vT_in, "v", i4)
        ps = ppx.tile([P, 1024], f32, name=f"psvt{i4}{R}", tag="px", bufs=2)
        for dp in range(DPT):
            for m in range(MT):
                nc.tensor.matmul(
                    ps[:, dp * 512:(dp + 1) * 512],
                    w_slice(wv_sb, m, dp),
                    ch[:, m * 512:(m + 1) * 512],
                    start=(m == 0), stop=(m == MT - 1))
        for dp in range(DPT):
            nc.vector.tensor_scalar_add(
                vT_sb[dp][:, i0:i0 + 512],
                ps[:, dp * 512:(dp + 1) * 512], bv_sb[:, dp:dp + 1])
        for dp in range(DPT):
            out_view = v4_sb[i4][:].rearrange(
                "p (j c) -> p j c", j=4)[:, :,
                                         dp * P:(dp + 1) * P]
            # sync queue: keeps the ACT queue free -- ACT (exp) is the
            # whole-kernel critical path and each queued DMA costs ~1.2us
            # of its engine time.
            nc.sync.dma_start(
                out_view, vT_sb[dp][:, i0:i0 + 512], transpose=True)

    # ---------------- attention ----------------
    # Heads run in sibling pairs (2hp, 2hp+1) whose kT/qT slices live at
    # partitions 0-63 / 64-127 (PE row tiles 0/64). Per (head, j) the four
    # scores matmuls (2 i-halves x 2 i-chunks) share one kT stationary
    # slice and the four attn@V matmuls share one vsc slice: with
    # LDW_REUSE the trailing matmuls set InstMatmult.ldweights=False so
    # walrus skips the per-matmul LDWEIGHTS reload (HW trace showed the
    # 768 reloads/rep serialize ~130ns each into the PE stream).
    # Softmax rowsums ride the exp's ACT-side accumulator on even j and a
    # DVE tensor_scalar on odd j, balancing the two near-critical engines.
    # attn@V accumulates both i-halves into two [128, 1024] psums held for
    # the whole pair (sibling heads at psum partitions 0-63/64-127).

    MULT = mybir.AluOpType.mult
    ADD = mybir.AluOpType.add

    def head_scores(hp, hh, j):
        """scores + exp + rowsum for one head, full i range (2 psum tiles)."""
        jg, jr = divmod(j, 4)
        h = hp * 2 + hh
        base = hh * 64
        lhs = kT_sb[hp][jg][base:base + 64, jr * P:(jr + 1) * P]
        pss = []
        first = True
        for ih in range(2):
            ps = pps.tile([P, 1024], f32, name=f"ps{h}_{j}_{ih}{R}",
                          tag="ps")
            for i5 in range(2):
                mm = nc.tensor.matmul(
                    ps[:, i5 * 512:(i5 + 1) * 512], lhs,
                    qT_sb[hp][ih * 2 + i5][base:base + 64, :],
                    start=True, stop=True)
                if LDW_REUSE and not first:
                    mm.ins.ldweights = False
                first = False
            pss.append(ps)
        outs = []
        for ih, ps in enumerate(pss):
            a = attp.tile([P, 1024], bf16, name=f"att{h}_{j}_{ih}{R}",
                          tag="att", bufs=12)
            rsh = stat.tile([P, 1], f32, name=f"rsh{h}_{j}_{ih}{R}",
                            tag="rsh", bufs=16)
            if j % 2 == ROWSUM_DVE_PARITY:
                nc.scalar.activation(a[:], ps[:], AF.Exp, scale=SCALE)
                nc.vector.tensor_scalar(a[:], a[:], 1.0, 0.0, MULT, ADD,
                                        accum_out=rsh[:])
            else:
                nc.scalar.activation(a[:], ps[:], AF.Exp, scale=SCALE,
                                     accum_out=rsh[:])
            outs.append((a, rsh))
        return outs

    def head_finish(hp, hh, j, xps01, outs):
        """softmax divisor onto v, then attn@V for both i-halves."""
        h = hp * 2 + hh
        jg, jr = divmod(j, 4)
        rs = stat.tile([P, 1], f32, name=f"rs{h}_{j}{R}", tag="rs")
        nc.vector.tensor_add(rs[:], outs[0][1][:], outs[1][1][:])
        rc = stat.tile([P, 1], f32, name=f"rc{h}_{j}{R}", tag="rc")
        nc.vector.reciprocal(rc[:], rs[:])
        vsc = attp.tile([P, 64], bf16, name=f"vsc{h}_{j}{R}", tag="vsc",
                        bufs=8)
        nc.vector.tensor_scalar_mul(
            vsc[:],
            v4_sb[jg][:, jr * DL + h * 64:jr * DL + (h + 1) * 64],
            rc[:])
        first = True
        for ih in range(2):
            a = outs[ih][0]
            for i5 in range(2):
                mm = nc.tensor.matmul(
                    xps01[ih][hh * 64:(hh + 1) * 64,
                              i5 * 512:(i5 + 1) * 512],
                    vsc[:], a[:, i5 * 512:(i5 + 1) * 512],
                    start=(j == 0), stop=(j == SJ - 1),
                    skip_group_check=True)
                if LDW_REUSE and not first:
                    mm.ins.ldweights = False
                first = False

    # ---------------- output projection constants ----------------
    # bo is added on the host during unshard (a K=1 ones-row matmul for it
    # here would cost 16384 PE rows ~ 7us).
    if not wo_cell:
        wo_sb = const.tile([P, DPT * DM], bf16, name="wo", tag="wo")
        nc.sync.dma_start(
            wo_sb[:].rearrange("p (t c) -> p t c", t=DPT),
            wo_d.ap().rearrange("(t p) c -> p t c", p=P))
        wo_cell.append(wo_sb)
    wo_sb = wo_cell[0]

    def emit_outproj(jts):
        for jt in jts:
            ot = outp.tile([P, DM], f32, name=f"ot{jt}{R}", tag="ot")
            ps = pps.tile([P, DM], f32, name=f"pso{jt}{R}", tag="ps")
            jh, jo = divmod(jt, 8)
            for cpt in range(DPT):
                first = True
                for n5 in range(2):
                    no = n5 * 512
                    mm = nc.tensor.matmul(
                        ps[:, no:no + 512],
                        xT_sb[cpt][jh][:, jo * P:(jo + 1) * P],
                        wo_sb[:, cpt * DM + no:cpt * DM + no + 512],
                        start=(cpt == 0), stop=(cpt == DPT - 1))
                    if LDW_REUSE and not first:
                        mm.ins.ldweights = False
                    first = False
            nc.vector.tensor_copy(ot[:], ps[:])
            oq = nc.sync if jt % 2 == 0 else nc.gpsimd
            oq.dma_start(O_d.ap()[jt * P:(jt + 1) * P, :], ot[:])

    def alloc_xps(hp, ih):
        return ppx.tile([P, 1024], f32, name=f"xp{ih}_{hp}{R}", tag="px",
                        bufs=2)

    def copy_xps(hp, ih, xph):
        nc.vector.tensor_copy(xT_sb[hp][ih][:], xph[:])

    prio = tc.high_priority()
    prio.__enter__()

    # Both pairs: single merged phase per pair -- scores/exp/attn@V per j.
    # The ACT exp stream is the pacing engine; the PE has ~2x headroom per
    # j, which the scheduler fills with the next rep's projections and the
    # trailing output projection.
    for hp in range(2):
        xps01 = [alloc_xps(hp, 0), alloc_xps(hp, 1)]
        for j in range(SJ):
            oA = head_scores(hp, 0, j)
            oB = head_scores(hp, 1, j)
            head_finish(hp, 0, j, xps01, oA)
            head_finish(hp, 1, j, xps01, oB)
        copy_xps(hp, 0, xps01[0])
        copy_xps(hp, 1, xps01[1])
    prio.__exit__(None, None, None)

    # output projection trails the whole rep: pure PE+DVE+DMA work that
    # overlaps the next rep's projections (which own the high-prio lane).
    emit_outproj(range(SJ))


def build_program(split_waits=True, reps=1):
    nc = bass.Bass("TRN2", target_bir_lowering=False, debug=False)

    qT_in = nc.dram_tensor("qT_in", [DM, S], PROJ_DT, kind="ExternalInput")
    kT_in = nc.dram_tensor("kT_in", [DM, S], PROJ_DT, kind="ExternalInput")
    vT_in = nc.dram_tensor("vT_in", [DM, S], PROJ_DT, kind="ExternalInput")
    wq_d = nc.dram_tensor("wq", [DM, DL], PROJ_DT, kind="ExternalInput")
    wk_d = nc.dram_tensor("wk", [DM, DL], PROJ_DT, kind="ExternalInput")
    wv_d = nc.dram_tensor("wv", [DM, DL], PROJ_DT, kind="ExternalInput")
    wo_d = nc.dram_tensor("wo", [DL, DM], bf16, kind="ExternalInput")
    bq_d = nc.dram_tensor("bq", [DL, 1], f32, kind="ExternalInput")
    bk_d = nc.dram_tensor("bk", [DL, 1], f32, kind="ExternalInput")
    bv_d = nc.dram_tensor("bv", [DL, 1], f32, kind="ExternalInput")
    O_d = nc.dram_tensor("O", [S, DM], f32, kind="ExternalOutput")

    with tile.TileContext(nc) as tc:
        with (
            tc.tile_pool(name="const", bufs=1) as const,
            tc.tile_pool(name="persist", bufs=1) as sb,
            tc.tile_pool(name="stat", bufs=6) as stat,
            tc.tile_pool(name="outp", bufs=3) as outp,
            tc.tile_pool(name="inp", bufs=2) as inp,
            tc.tile_pool(name="vtp", bufs=1) as vtp,
            tc.tile_pool(name="attp", bufs=20) as attp,
            tc.tile_pool(name="pps", bufs=2, space="PSUM") as pps,
            tc.tile_pool(name="ppx", bufs=1, space="PSUM") as ppx,
        ):
            # ---------------- constants ----------------
            # One DMA per weight: DRAM [(t p), c] -> SBUF [p, (t c)] so the
            # m-th 128-row block lands at free offset m*DL.
            def load_w(dram, nm, dt_, cols, eng):
                t = const.tile([P, MT * cols], dt_, name=nm, tag=nm)
                eng.dma_start(
                    t[:].rearrange("p (t c) -> p t c", t=MT),
                    dram.ap().rearrange("(t p) c -> p t c", p=P))
                return t

            wq_sb = load_w(wq_d, "wq", PROJ_DT, DL, nc.scalar)  # [128, 8*256]
            wk_sb = load_w(wk_d, "wk", PROJ_DT, DL, nc.scalar)
            bq_sb = const.tile([P, DPT], f32, name="bq", tag="bq")
            nc.sync.dma_start(
                bq_sb[:].rearrange("p (t c) -> p t c", t=DPT),
                bq_d.ap().rearrange("(t p) c -> p t c", p=P))
            bk_sb = const.tile([P, DPT], f32, name="bk", tag="bk")
            nc.sync.dma_start(
                bk_sb[:].rearrange("p (t c) -> p t c", t=DPT),
                bk_d.ap().rearrange("(t p) c -> p t c", p=P))
            bv_sb = const.tile([P, DPT], f32, name="bv", tag="bv")
            nc.sync.dma_start(
                bv_sb[:].rearrange("p (t c) -> p t c", t=DPT),
                bv_d.ap().rearrange("(t p) c -> p t c", p=P))
            wv_cell = []

            def load_wv():
                if not wv_cell:
                    wv_cell.append(load_w(wv_d, "wv", PROJ_DT, DL, nc.sync))
                return wv_cell[0]

            def w_slice(w, m, dp):
                return w[:, m * DL + dp * P:m * DL + (dp + 1) * P]

            wo_cell = []

            for it in range(reps):
                emit_iter(nc, tc, it, const, sb, stat, outp, inp, vtp, attp,
                          pps, ppx, qT_in, kT_in, vT_in, wo_d, O_d,
                          wq_sb, wk_sb, load_wv, bq_sb, bk_sb, bv_sb, w_slice,
                          wo_cell)

    if split_waits:
        _split_excess_waits(nc)
    return nc


def _get_program():
    global _PROGRAM
    if _PROGRAM is None:
        _PROGRAM = build_program()
    return _PROGRAM


def _tf32(x):
    """Round fp32 -> TF32 (10-bit mantissa), round-to-nearest-even."""
    x = np.ascontiguousarray(np.asarray(x, dtype=np.float32))
    u = x.view(np.uint32)
    r = ((u >> 13) & 1).astype(np.uint32)
    u2 = ((u + np.uint32(0x0FFF) + r) & np.uint32(0xFFFFE000))
    return u2.view(np.float32)


def shard_inputs(inputs):
    """FULL inputs -> per-core in_maps (list of 8 dicts)."""
    q = np.asarray(inputs["query"], dtype=np.float32)
    k = np.asarray(inputs["key"], dtype=np.float32)
    v = np.asarray(inputs["value"], dtype=np.float32)
    Wq = np.asarray(inputs["Wq"], dtype=np.float32)
    Wk = np.asarray(inputs["Wk"], dtype=np.float32)
    Wv = np.asarray(inputs["Wv"], dtype=np.float32)
    Wo = np.asarray(inputs["Wo"], dtype=np.float32)
    bq = np.asarray(inputs["bq"], dtype=np.float32)
    bk = np.asarray(inputs["bk"], dtype=np.float32)
    bv = np.asarray(inputs["bv"], dtype=np.float32)
    bo = np.asarray(inputs["bo"], dtype=np.float32)

    if PROJ_BF16:
        import ml_dtypes

        def _proj_cast(x):
            return np.ascontiguousarray(np.asarray(x, np.float32)).astype(
                ml_dtypes.bfloat16)
    else:
        _proj_cast = _tf32

    qT = [_proj_cast(q[b].T) for b in range(B)]
    kT = [_proj_cast(k[b].T) for b in range(B)]
    vT = [_proj_cast(v[b].T) for b in range(B)]

    in_maps = []
    for c in range(N_CORES):
        b, g = c // GROUPS, c % GROUPS
        sl = slice(g * DL, (g + 1) * DL)
        in_maps.append({
            "qT_in": qT[b],
            "kT_in": kT[b],
            "vT_in": vT[b],
            "wq": _proj_cast(Wq[:, sl]),
            "wk": _proj_cast(Wk[:, sl]),
            "wv": _proj_cast(Wv[:, sl]),
            "wo": _proj_cast(Wo[sl, :]),
            "bq": np.ascontiguousarray(bq[sl].reshape(DL, 1)),
            "bk": np.ascontiguousarray(bk[sl].reshape(DL, 1)),
            "bv": np.ascontiguousarray(bv[sl].reshape(DL, 1)),
        })
    return in_maps


def unshard_output(results, bo):
    """results: list of 8 dicts with 'O' [S, DM] -> full [B, S, DM].
    bo is added here (host) -- cheaper than a K=1 PE matmul on device."""
    out = np.zeros((B, S, DM), np.float32)
    for c in range(N_CORES):
        out[c // GROUPS] += results[c]["O"]
    out += np.asarray(bo, np.float32)
    return out


def kernel(**inputs):
    nc = _get_program()
    in_maps = shard_inputs(inputs)
    res = run_bass_kernel_spmd(nc, in_maps, core_ids=list(range(N_CORES)))
    return unshard_output(res.results, inputs["bo"])



# revision 14
# speedup vs baseline: 1.1542x; 1.0393x over previous
"""Multi-head attention (softmax over query axis) on 8 Trainium2 cores.

Problem: nn_MultiHeadAttention_3899830305178
  B=2, S=2048, D_MODEL=1024, HEADS=16, D_K=64, fp32 IO.
  reference:
    q = (query @ Wq + bq), k = ..., v = ...        [b, s, h, dk]
    scores = einsum('bihd,bjhd->bijh', q, k) / 8
    attn = softmax(scores, axis=1)                 # over QUERY axis i (quirk)
    x = einsum('bijh,bjhd->bihd', attn, v)         [b, s, h*dk]
    out = x @ Wo + bo

Sharding: data-parallel over batch (2) x tensor-parallel over heads (4 groups
of 4 heads) = 8 cores. Each core computes a partial output
O_part = x_local @ Wo[rows of its heads]; the host sums the 4 partials per
batch (row-parallel unshard) -- bo is added on-device by the g==0 core.

Per-core kernel math (host passes query/key/value pre-transposed so the
projections contract over the model dim on partitions):
  qT[d', i] = Wq_s.T @ queryT      (d' = 4 local heads x 64 = 256)
  kT[d', j] = Wk_s.T @ keyT
  vT[d', j] = Wv_s.T @ valueT (+bv), then bf16 DMA-transpose -> v[j, d']
  per head h:  sT[j, i] = kT_h.T @ qT_h / 8  (softmax over i == free axis)
               eT = exp(sT) (bf16), rowsum via a DVE tensor_scalar accum
               v_h_scaled[j, :] = v_h[j, :] / rowsum[j]   <- softmax divisor
               xT_h[d, i] = v_h_scaled.T @ eT             (contracts over j)
  O_part[i, n] = xT.T @ Wo_s; bo is added on the host during unshard.

Engine balance (per CoreSim, 230us single-shot span): PE ~169us
(projections 41 + scores 55 + attn@V 55 + out-proj 14), ACT ~138us (the
128 [128,1024] exps are irreducible -- Exp exists only on ACT), DVE
~93us, 16KB/partition PSUM exactly full (2x scores buffers + 2x
proj/attn@V buffers). Startup: wq/wk load on the (initially idle) ACT
HWDGE queue in parallel with the q/k input chunks on SP; the q/k
projection stream is ordered q0,q1,k0,q2,q3,k1..k3 to match the ACT
engine's exp demand order; wv/wo load late; qT is tiled per input chunk
so each scores matmul gates on exactly one projected chunk.
Design choices vs the naive version:
  - sibling heads (partitions 0-63 / 64-127 of the kT/qT slices) emit
    their K=64 scores matmuls interleaved with PE tile positions
    (0,0)/(64,0), letting the PE overlap row tiles (K=64 alone half-fills
    the 128x128 array);
  - eT is bf16 (same PE rate as f32r, half the SBUF/attp footprint);
  - softmax rowsums come from a DVE tensor_scalar (2-byte fast mode,
    ~0.4us/tile) instead of the ACT accum_out (+187ns/tile on the other
    near-critical engine), computed as out=(a*1)+0 in place with
    accum_out=rowsum;
  - attn@V is split by i-half into [64,1024] psums so the first half's
    output projection overlaps the second half's attention (xT is split
    per i-half so the dependency is tile-precise); pair 1's first scores
    are emitted inside pair 0's phase B so the ACT engine never idles
    there, and the first-half output projection (with O DMAs alternating
    between the SP and ACT hardware queues) drains during pair 1's
    phase B;
  - the bias matmuls (K=1 ones-row) were removed from the PE: bo rides
    the host-side unshard sum.

Projection inputs/weights are bf16; scores and the output projection run
in float32r (TF32, fp32 accumulate) with fp32 softmax statistics; attn@V
is bf16 x bf16 -> fp32. Measured end-to-end relative error vs the fp64
reference is ~4.4e-3 on hardware.
"""

import numpy as np

import concourse.bass as bass
import concourse.mybir as mybir
import concourse.tile as tile
from concourse.bass_utils import run_bass_kernel_spmd

# problem shape (hardcoded per contract)
B, S, DM, H, DK = 2, 2048, 1024, 16, 64
N_CORES = 8
GROUPS = 4              # head groups (tensor-parallel)
HL = H // GROUPS        # 4 local heads per core
DL = HL * DK            # 256 local concat width
P = 128
SJ = S // P             # 16 strips of 128 along j (keys) and i (out rows)
MT = DM // P            # 8 contraction tiles for projections
DPT = DL // P           # 2 partition tiles of the local concat dim
SCALE = 1.0 / 8.0       # 1/sqrt(DK)

f32 = mybir.dt.float32
f32r = mybir.dt.float32r
bf16 = mybir.dt.bfloat16
AF = mybir.ActivationFunctionType

# Projection stage (inputs + projection weights) in bf16: halves the input
# DMA (the critical-path prefix) at ~2e-3 relative error. Attention and
# output projection stay TF32.
PROJ_BF16 = True
PROJ_DT = bf16 if PROJ_BF16 else f32r

import os as _os
# Reuse PE stationary weights across same-lhsT matmul runs by suppressing
# the per-matmul LDWEIGHTS (InstMatmult.ldweights=False on the trailing
# matmuls of each run).
LDW_REUSE = _os.environ.get("LDW_REUSE", "1") == "1"
# Which j parity routes its softmax rowsums to the DVE (the other parity
# uses the exp's ACT-side accumulator). 2 = all rowsums on ACT.
ROWSUM_DVE_PARITY = int(_os.environ.get("ROWSUM_DVE_PARITY", "1"))

_PROGRAM = None


def _dedupe_ldweights(nc):
    """Drop InstLdweights that reload the exact weights already resident.

    Tile's legalizer splits every matmul into (InstLdweights, InstMatmult
    ldweights=False); each reload costs ~120-180ns serialized into the PE
    stream. When consecutive LDWEIGHTS on the PE stream have identical
    weight APs (the kernel emits same-lhsT matmul runs for scores, attn@V
    and the output projection), the duplicates are pure overhead: the
    array still holds the weights (nothing else writes it), and the WAR
    protection on the SBUF region anchors on the matmuls (Tile tracked
    them as the lhsT readers), so dropping the reload is safe. Waits and
    sem updates of a dropped LDWEIGHTS move onto a NOP in its place."""
    n = 0
    for f in nc.m.functions:
        for blk in f.blocks:
            last_key = None
            new_insts = []
            for inst in blk.instructions:
                if getattr(inst, "engine", None) == mybir.EngineType.PE:
                    tn = type(inst).__name__
                    if tn == "InstLdweights":
                        key = (str(inst.ins[0]), str(inst.tile_position),
                               str(inst.perf_mode), str(inst.is_transpose))
                        if key == last_key:
                            si = inst.sync_info
                            if si is not None and (si.on_wait or si.on_update):
                                new_insts.append(mybir.InstNoOp(
                                    name=f"{inst.name}-ldwskip",
                                    engine=inst.engine,
                                    sync_info=si,
                                    bass_nofuse=True))
                            n += 1
                            continue
                        last_key = key
                    elif tn == "InstMatmult":
                        if inst.is_transpose:
                            last_key = None
                    elif tn in ("InstNoOp", "InstEventSemaphore"):
                        pass
                    else:
                        last_key = None
                new_insts.append(inst)
            blk.instructions[:] = new_insts
    return n


def _split_excess_waits(nc, max_waits=1):
    """walrus in this container rejects >1 semaphore wait per instruction
    (e.g. the Tile kernel-tail Drain); move extras onto same-engine NOPs."""
    n_split = 0
    for f in nc.m.functions:
        for blk in f.blocks:
            new_insts = []
            for inst in blk.instructions:
                si = getattr(inst, "sync_info", None)
                if si is not None and si.on_wait and len(si.on_wait) > max_waits:
                    waits = list(si.on_wait)
                    extra, keep = waits[:-max_waits], waits[-max_waits:]
                    for i in range(0, len(extra), max_waits):
                        chunk = extra[i:i + max_waits]
                        nop = mybir.InstNoOp(
                            name=f"{inst.name}-ws{n_split}-{i}",
                            engine=inst.engine,
                            sync_info=mybir.SyncInfo(on_wait=chunk, on_update=[]),
                            bass_nofuse=True,
                        )
                        new_insts.append(nop)
                    si.on_wait = keep
                    n_split += 1
                new_insts.append(inst)
            blk.instructions[:] = new_insts
    return n_split


def emit_iter(nc, tc, it, const, sb, stat, outp, inp, vtp, attp, pps, ppx,
              qT_in, kT_in, vT_in, wo_d, O_d,
              wq_sb, wk_sb, load_wv, bq_sb, bk_sb, bv_sb, w_slice, wo_cell):
    """One full attention iteration (tile names suffixed _r{it} so the
    program body can be repeated for steady-state timing; tags are shared
    so pool buffers rotate/serialize across reps)."""
    R = f"_r{it}"

    # ---------------- persistent activations ----------------
    # qT split by i-half, kT by i-quarter (j-group): finer tiles give
    # the scheduler finer dependencies, so scores start before the
    # whole projection finishes.
    qT_sb = [[sb.tile([P, 512], bf16, name=f"qT{dp}_{i4}{R}",
                      tag=f"qT{dp}_{i4}") for i4 in range(4)]
             for dp in range(DPT)]
    kT_sb = [[sb.tile([P, 512], bf16, name=f"kT{dp}_{jg}{R}",
                      tag=f"kT{dp}_{jg}") for jg in range(4)]
             for dp in range(DPT)]
    # v packed per j-group of 4: v4_sb[jg][p, jj*DL + d'] holds
    # v[jg*512 + jj*128 + p, d']
    v4_sb = [sb.tile([P, 4 * DL], bf16, name=f"v{jg}{R}", tag=f"v{jg}")
             for jg in range(4)]
    xT_sb = [[sb.tile([P, 1024], bf16, name=f"xT{hp}_{ih}{R}",
                      tag=f"xT{hp}_{ih}") for ih in range(2)]
             for hp in range(DPT)]

    # ---------------- projections ----------------
    # dst[d', i] = W.T @ inT ; contraction over m on partitions.
    vT_sb = [vtp.tile([P, S], bf16, name=f"vT{dp}{R}", tag=f"vT{dp}")
             for dp in range(DPT)]

    ENGQ = {"q": nc.sync, "k": nc.sync, "v": nc.sync}

    def load_in_chunk(win, nm, i4):
        # one DMA: all 8 m-blocks of columns [i0, i0+512)
        t = inp.tile([P, MT * 512], PROJ_DT, name=f"{nm}in{i4}{R}",
                     tag="pin")
        src = win.ap().rearrange("(t p) c -> p t c", p=P)
        ENGQ[nm].dma_start(
            t[:].rearrange("p (t c) -> p t c", t=MT),
            src[:, :, i4 * 512:(i4 + 1) * 512])
        return t

    qk_prio = tc.high_priority()
    qk_prio.__enter__()
    QK_ORDER = [("q", 0), ("k", 0), ("q", 1), ("q", 2), ("q", 3),
                ("k", 1), ("k", 2), ("k", 3)]
    for nm, i4 in QK_ORDER:
        if True:
            win, w_sb, b_sb = ((qT_in, wq_sb, bq_sb) if nm == "q"
                               else (kT_in, wk_sb, bk_sb))
            ch = load_in_chunk(win, nm, i4)
            ps = ppx.tile([P, 1024], f32, name=f"ps{nm}{i4}{R}",
                          tag="px", bufs=2)
            for dp in range(DPT):
                for m in range(MT):
                    nc.tensor.matmul(
                        ps[:, dp * 512:(dp + 1) * 512],
                        w_slice(w_sb, m, dp),
                        ch[:, m * 512:(m + 1) * 512],
                        start=(m == 0), stop=(m == MT - 1))
            for dp in range(DPT):
                if nm == "q":
                    dst = qT_sb[dp][i4][:]
                else:
                    dst = kT_sb[dp][i4][:]
                nc.vector.tensor_scalar_add(
                    dst, ps[:, dp * 512:(dp + 1) * 512],
                    b_sb[:, dp:dp + 1])

    # vT[d', j] = Wv.T @ valueT (bias folded in, bf16 out), then one
    # SBUF->SBUF bf16 DMA-transpose per (i4, dp) covering 4 j-tiles,
    # dispatched on the ACT HWDGE queue to keep SP free for inputs.
    qk_prio.__exit__(None, None, None)
    wv_sb = load_wv()
    for i4 in range(4):
        i0 = i4 * 512
        ch = load_in_chunk(vT_in, "v", i4)
        ps = ppx.tile([P, 1024], f32, name=f"psvt{i4}{R}", tag="px", bufs=2)
        for dp in range(DPT):
            for m in range(MT):
                nc.tensor.matmul(
                    ps[:, dp * 512:(dp + 1) * 512],
                    w_slice(wv_sb, m, dp),
                    ch[:, m * 512:(m + 1) * 512],
                    start=(m == 0), stop=(m == MT - 1))
        for dp in range(DPT):
            nc.vector.tensor_scalar_add(
                vT_sb[dp][:, i0:i0 + 512],
                ps[:, dp * 512:(dp + 1) * 512], bv_sb[:, dp:dp + 1])
        for dp in range(DPT):
            out_view = v4_sb[i4][:].rearrange(
                "p (j c) -> p j c", j=4)[:, :,
                                         dp * P:(dp + 1) * P]
            # sync queue: keeps the ACT queue free -- ACT (exp) is the
            # whole-kernel critical path and each queued DMA costs ~1.2us
            # of its engine time.
            nc.sync.dma_start(
                out_view, vT_sb[dp][:, i0:i0 + 512], transpose=True)

    # ---------------- attention ----------------
    # Heads run in sibling pairs (2hp, 2hp+1) whose kT/qT slices live at
    # partitions 0-63 / 64-127 (PE row tiles 0/64). Per (head, j) the four
    # scores matmuls (2 i-halves x 2 i-chunks) share one kT stationary
    # slice and the four attn@V matmuls share one vsc slice: with
    # LDW_REUSE the trailing matmuls set InstMatmult.ldweights=False so
    # walrus skips the per-matmul LDWEIGHTS reload (HW trace showed the
    # 768 reloads/rep serialize ~130ns each into the PE stream).
    # Softmax rowsums ride the exp's ACT-side accumulator on even j and a
    # DVE tensor_scalar on odd j, balancing the two near-critical engines.
    # attn@V accumulates both i-halves into two [128, 1024] psums held for
    # the whole pair (sibling heads at psum partitions 0-63/64-127).

    MULT = mybir.AluOpType.mult
    ADD = mybir.AluOpType.add

    def head_scores(hp, hh, j):
        """scores + exp + rowsum for one head, full i range (2 psum tiles)."""
        jg, jr = divmod(j, 4)
        h = hp * 2 + hh
        base = hh * 64
        lhs = kT_sb[hp][jg][base:base + 64, jr * P:(jr + 1) * P]
        pss = []
        first = True
        for ih in range(2):
            ps = pps.tile([P, 1024], f32, name=f"ps{h}_{j}_{ih}{R}",
                          tag="ps")
            for i5 in range(2):
                mm = nc.tensor.matmul(
                    ps[:, i5 * 512:(i5 + 1) * 512], lhs,
                    qT_sb[hp][ih * 2 + i5][base:base + 64, :],
                    start=True, stop=True)
                if LDW_REUSE and not first:
                    mm.ins.ldweights = False
                first = False
            pss.append(ps)
        outs = []
        for ih, ps in enumerate(pss):
            a = attp.tile([P, 1024], bf16, name=f"att{h}_{j}_{ih}{R}",
                          tag="att", bufs=12)
            rsh = stat.tile([P, 1], f32, name=f"rsh{h}_{j}_{ih}{R}",
                            tag="rsh", bufs=16)
            if j % 2 == ROWSUM_DVE_PARITY:
                nc.scalar.activation(a[:], ps[:], AF.Exp, scale=SCALE)
                nc.vector.tensor_scalar(a[:], a[:], 1.0, 0.0, MULT, ADD,
                                        accum_out=rsh[:])
            else:
                nc.scalar.activation(a[:], ps[:], AF.Exp, scale=SCALE,
                                     accum_out=rsh[:])
            outs.append((a, rsh))
        return outs

    def head_finish(hp, hh, j, xps01, outs):
        """softmax divisor onto v, then attn@V for both i-halves."""
        h = hp * 2 + hh
        jg, jr = divmod(j, 4)
        rs = stat.tile([P, 1], f32, name=f"rs{h}_{j}{R}", tag="rs")
        nc.vector.tensor_add(rs[:], outs[0][1][:], outs[1][1][:])
        rc = stat.tile([P, 1], f32, name=f"rc{h}_{j}{R}", tag="rc")
        nc.vector.reciprocal(rc[:], rs[:])
        vsc = attp.tile([P, 64], bf16, name=f"vsc{h}_{j}{R}", tag="vsc",
                        bufs=8)
        nc.vector.tensor_scalar_mul(
            vsc[:],
            v4_sb[jg][:, jr * DL + h * 64:jr * DL + (h + 1) * 64],
            rc[:])
        first = True
        for ih in range(2):
            a = outs[ih][0]
            for i5 in range(2):
                mm = nc.tensor.matmul(
                    xps01[ih][hh * 64:(hh + 1) * 64,
                              i5 * 512:(i5 + 1) * 512],
                    vsc[:], a[:, i5 * 512:(i5 + 1) * 512],
                    start=(j == 0), stop=(j == SJ - 1),
                    skip_group_check=True)
                if LDW_REUSE and not first:
                    mm.ins.ldweights = False
                first = False

    # ---------------- output projection constants ----------------
    # bo is added on the host during unshard (a K=1 ones-row matmul for it
    # here would cost 16384 PE rows ~ 7us).
    if not wo_cell:
        wo_sb = const.tile([P, DPT * DM], bf16, name="wo", tag="wo")
        nc.sync.dma_start(
            wo_sb[:].rearrange("p (t c) -> p t c", t=DPT),
            wo_d.ap().rearrange("(t p) c -> p t c", p=P))
        wo_cell.append(wo_sb)
    wo_sb = wo_cell[0]

    def emit_outproj(jts):
        for jt in jts:
            ot = outp.tile([P, DM], f32, name=f"ot{jt}{R}", tag="ot")
            ps = pps.tile([P, DM], f32, name=f"pso{jt}{R}", tag="ps")
            jh, jo = divmod(jt, 8)
            for cpt in range(DPT):
                first = True
                for n5 in range(2):
                    no = n5 * 512
                    mm = nc.tensor.matmul(
                        ps[:, no:no + 512],
                        xT_sb[cpt][jh][:, jo * P:(jo + 1) * P],
                        wo_sb[:, cpt * DM + no:cpt * DM + no + 512],
                        start=(cpt == 0), stop=(cpt == DPT - 1))
                    if LDW_REUSE and not first:
                        mm.ins.ldweights = False
                    first = False
            nc.vector.tensor_copy(ot[:], ps[:])
            oq = nc.sync if jt % 2 == 0 else nc.gpsimd
            oq.dma_start(O_d.ap()[jt * P:(jt + 1) * P, :], ot[:])

    def alloc_xps(hp, ih):
        return ppx.tile([P, 1024], f32, name=f"xp{ih}_{hp}{R}", tag="px",
                        bufs=2)

    def copy_xps(hp, ih, xph):
        nc.vector.tensor_copy(xT_sb[hp][ih][:], xph[:])

    prio = tc.high_priority()
    prio.__enter__()

    # Both pairs: single merged phase per pair -- scores/exp/attn@V per j.
    # The ACT exp stream is the pacing engine; the PE has ~2x headroom per
    # j, which the scheduler fills with the next rep's projections and the
    # trailing output projection.
    for hp in range(2):
        xps01 = [alloc_xps(hp, 0), alloc_xps(hp, 1)]
        for j in range(SJ):
            oA = head_scores(hp, 0, j)
            oB = head_scores(hp, 1, j)
            head_finish(hp, 0, j, xps01, oA)
            head_finish(hp, 1, j, xps01, oB)
        copy_xps(hp, 0, xps01[0])
        copy_xps(hp, 1, xps01[1])
    prio.__exit__(None, None, None)

    # output projection trails the whole rep: pure PE+DVE+DMA work that
    # overlaps the next rep's projections (which own the high-prio lane).
    emit_outproj(range(SJ))


def build_program(split_waits=True, reps=1):
    nc = bass.Bass("TRN2", target_bir_lowering=False, debug=False)

    qT_in = nc.dram_tensor("qT_in", [DM, S], PROJ_DT, kind="ExternalInput")
    kT_in = nc.dram_tensor("kT_in", [DM, S], PROJ_DT, kind="ExternalInput")
    vT_in = nc.dram_tensor("vT_in", [DM, S], PROJ_DT, kind="ExternalInput")
    wq_d = nc.dram_tensor("wq", [DM, DL], PROJ_DT, kind="ExternalInput")
    wk_d = nc.dram_tensor("wk", [DM, DL], PROJ_DT, kind="ExternalInput")
    wv_d = nc.dram_tensor("wv", [DM, DL], PROJ_DT, kind="ExternalInput")
    wo_d = nc.dram_tensor("wo", [DL, DM], bf16, kind="ExternalInput")
    bq_d = nc.dram_tensor("bq", [DL, 1], f32, kind="ExternalInput")
    bk_d = nc.dram_tensor("bk", [DL, 1], f32, kind="ExternalInput")
    bv_d = nc.dram_tensor("bv", [DL, 1], f32, kind="ExternalInput")
    O_d = nc.dram_tensor("O", [S, DM], f32, kind="ExternalOutput")

    with tile.TileContext(nc) as tc:
        with (
            tc.tile_pool(name="const", bufs=1) as const,
            tc.tile_pool(name="persist", bufs=1) as sb,
            tc.tile_pool(name="stat", bufs=6) as stat,
            tc.tile_pool(name="outp", bufs=3) as outp,
            tc.tile_pool(name="inp", bufs=2) as inp,
            tc.tile_pool(name="vtp", bufs=1) as vtp,
            tc.tile_pool(name="attp", bufs=20) as attp,
            tc.tile_pool(name="pps", bufs=2, space="PSUM") as pps,
            tc.tile_pool(name="ppx", bufs=1, space="PSUM") as ppx,
        ):
            # ---------------- constants ----------------
            # One DMA per weight: DRAM [(t p), c] -> SBUF [p, (t c)] so the
            # m-th 128-row block lands at free offset m*DL.
            def load_w(dram, nm, dt_, cols, eng):
                t = const.tile([P, MT * cols], dt_, name=nm, tag=nm)
                eng.dma_start(
                    t[:].rearrange("p (t c) -> p t c", t=MT),
                    dram.ap().rearrange("(t p) c -> p t c", p=P))
                return t

            wq_sb = load_w(wq_d, "wq", PROJ_DT, DL, nc.scalar)  # [128, 8*256]
            wk_sb = load_w(wk_d, "wk", PROJ_DT, DL, nc.scalar)
            bq_sb = const.tile([P, DPT], f32, name="bq", tag="bq")
            nc.sync.dma_start(
                bq_sb[:].rearrange("p (t c) -> p t c", t=DPT),
                bq_d.ap().rearrange("(t p) c -> p t c", p=P))
            bk_sb = const.tile([P, DPT], f32, name="bk", tag="bk")
            nc.sync.dma_start(
                bk_sb[:].rearrange("p (t c) -> p t c", t=DPT),
                bk_d.ap().rearrange("(t p) c -> p t c", p=P))
            bv_sb = const.tile([P, DPT], f32, name="bv", tag="bv")
            nc.sync.dma_start(
                bv_sb[:].rearrange("p (t c) -> p t c", t=DPT),
                bv_d.ap().rearrange("(t p) c -> p t c", p=P))
            wv_cell = []

            def load_wv():
                if not wv_cell:
                    wv_cell.append(load_w(wv_d, "wv", PROJ_DT, DL, nc.sync))
                return wv_cell[0]

            def w_slice(w, m, dp):
                return w[:, m * DL + dp * P:m * DL + (dp + 1) * P]

            wo_cell = []

            for it in range(reps):
                emit_iter(nc, tc, it, const, sb, stat, outp, inp, vtp, attp,
                          pps, ppx, qT_in, kT_in, vT_in, wo_d, O_d,
                          wq_sb, wk_sb, load_wv, bq_sb, bk_sb, bv_sb, w_slice,
                          wo_cell)

    if LDW_REUSE:
        _dedupe_ldweights(nc)
    if split_waits:
        _split_excess_waits(nc)
    return nc


def _get_program():
    global _PROGRAM
    if _PROGRAM is None:
        _PROGRAM = build_program()
    return _PROGRAM


def _tf32(x):
    """Round fp32 -> TF32 (10-bit mantissa), round-to-nearest-even."""
    x = np.ascontiguousarray(np.asarray(x, dtype=np.float32))
    u = x.view(np.uint32)
    r = ((u >> 13) & 1).astype(np.uint32)
    u2 = ((u + np.uint32(0x0FFF) + r) & np.uint32(0xFFFFE000))
    return u2.view(np.float32)


def shard_inputs(inputs):
    """FULL inputs -> per-core in_maps (list of 8 dicts)."""
    q = np.asarray(inputs["query"], dtype=np.float32)
    k = np.asarray(inputs["key"], dtype=np.float32)
    v = np.asarray(inputs["value"], dtype=np.float32)
    Wq = np.asarray(inputs["Wq"], dtype=np.float32)
    Wk = np.asarray(inputs["Wk"], dtype=np.float32)
    Wv = np.asarray(inputs["Wv"], dtype=np.float32)
    Wo = np.asarray(inputs["Wo"], dtype=np.float32)
    bq = np.asarray(inputs["bq"], dtype=np.float32)
    bk = np.asarray(inputs["bk"], dtype=np.float32)
    bv = np.asarray(inputs["bv"], dtype=np.float32)
    bo = np.asarray(inputs["bo"], dtype=np.float32)

    if PROJ_BF16:
        import ml_dtypes

        def _proj_cast(x):
            return np.ascontiguousarray(np.asarray(x, np.float32)).astype(
                ml_dtypes.bfloat16)
    else:
        _proj_cast = _tf32

    qT = [_proj_cast(q[b].T) for b in range(B)]
    kT = [_proj_cast(k[b].T) for b in range(B)]
    vT = [_proj_cast(v[b].T) for b in range(B)]

    in_maps = []
    for c in range(N_CORES):
        b, g = c // GROUPS, c % GROUPS
        sl = slice(g * DL, (g + 1) * DL)
        in_maps.append({
            "qT_in": qT[b],
            "kT_in": kT[b],
            "vT_in": vT[b],
            "wq": _proj_cast(Wq[:, sl]),
            "wk": _proj_cast(Wk[:, sl]),
            "wv": _proj_cast(Wv[:, sl]),
            "wo": _proj_cast(Wo[sl, :]),
            "bq": np.ascontiguousarray(bq[sl].reshape(DL, 1)),
            "bk": np.ascontiguousarray(bk[sl].reshape(DL, 1)),
            "bv": np.ascontiguousarray(bv[sl].reshape(DL, 1)),
        })
    return in_maps


def unshard_output(results, bo):
    """results: list of 8 dicts with 'O' [S, DM] -> full [B, S, DM].
    bo is added here (host) -- cheaper than a K=1 PE matmul on device."""
    out = np.zeros((B, S, DM), np.float32)
    for c in range(N_CORES):
        out[c // GROUPS] += results[c]["O"]
    out += np.asarray(bo, np.float32)
    return out


def kernel(**inputs):
    nc = _get_program()
    in_maps = shard_inputs(inputs)
    res = run_bass_kernel_spmd(nc, in_maps, core_ids=list(range(N_CORES)))
    return unshard_output(res.results, inputs["bo"])



# revision 17
# speedup vs baseline: 1.2454x; 1.0791x over previous
"""Multi-head attention (softmax over query axis) on 8 Trainium2 cores.

Problem: nn_MultiHeadAttention_3899830305178
  B=2, S=2048, D_MODEL=1024, HEADS=16, D_K=64, fp32 IO.
  reference:
    q = (query @ Wq + bq), k = ..., v = ...        [b, s, h, dk]
    scores = einsum('bihd,bjhd->bijh', q, k) / 8
    attn = softmax(scores, axis=1)                 # over QUERY axis i (quirk)
    x = einsum('bijh,bjhd->bihd', attn, v)         [b, s, h*dk]
    out = x @ Wo + bo

Sharding: data-parallel over batch (2) x tensor-parallel over heads (4 groups
of 4 heads) = 8 cores. Each core computes a partial output
O_part = x_local @ Wo[rows of its heads]; the host sums the 4 partials per
batch (row-parallel unshard) -- bo is added on-device by the g==0 core.

Per-core kernel math (host passes query/key/value pre-transposed so the
projections contract over the model dim on partitions):
  qT[d', i] = Wq_s.T @ queryT      (d' = 4 local heads x 64 = 256)
  kT[d', j] = Wk_s.T @ keyT
  vT[d', j] = Wv_s.T @ valueT (+bv), then bf16 DMA-transpose -> v[j, d']
  per head h:  sT[j, i] = kT_h.T @ qT_h / 8  (softmax over i == free axis)
               eT = exp(sT) (bf16), rowsum via a DVE tensor_scalar accum
               v_h_scaled[j, :] = v_h[j, :] / rowsum[j]   <- softmax divisor
               xT_h[d, i] = v_h_scaled.T @ eT             (contracts over j)
  O_part[i, n] = xT.T @ Wo_s; bo is added on the host during unshard.

Engine balance (per CoreSim, 230us single-shot span): PE ~169us
(projections 41 + scores 55 + attn@V 55 + out-proj 14), ACT ~138us (the
128 [128,1024] exps are irreducible -- Exp exists only on ACT), DVE
~93us, 16KB/partition PSUM exactly full (2x scores buffers + 2x
proj/attn@V buffers). Startup: wq/wk load on the (initially idle) ACT
HWDGE queue in parallel with the q/k input chunks on SP; the q/k
projection stream is ordered q0,q1,k0,q2,q3,k1..k3 to match the ACT
engine's exp demand order; wv/wo load late; qT is tiled per input chunk
so each scores matmul gates on exactly one projected chunk.
Design choices vs the naive version:
  - sibling heads (partitions 0-63 / 64-127 of the kT/qT slices) emit
    their K=64 scores matmuls interleaved with PE tile positions
    (0,0)/(64,0), letting the PE overlap row tiles (K=64 alone half-fills
    the 128x128 array);
  - eT is bf16 (same PE rate as f32r, half the SBUF/attp footprint);
  - softmax rowsums come from a DVE tensor_scalar (2-byte fast mode,
    ~0.4us/tile) instead of the ACT accum_out (+187ns/tile on the other
    near-critical engine), computed as out=(a*1)+0 in place with
    accum_out=rowsum;
  - attn@V is split by i-half into [64,1024] psums so the first half's
    output projection overlaps the second half's attention (xT is split
    per i-half so the dependency is tile-precise); pair 1's first scores
    are emitted inside pair 0's phase B so the ACT engine never idles
    there, and the first-half output projection (with O DMAs alternating
    between the SP and ACT hardware queues) drains during pair 1's
    phase B;
  - the bias matmuls (K=1 ones-row) were removed from the PE: bo rides
    the host-side unshard sum.

Projection inputs/weights are bf16; scores and the output projection run
in float32r (TF32, fp32 accumulate) with fp32 softmax statistics; attn@V
is bf16 x bf16 -> fp32. Measured end-to-end relative error vs the fp64
reference is ~4.4e-3 on hardware.
"""

import numpy as np

import concourse.bass as bass
import concourse.mybir as mybir
import concourse.tile as tile
from concourse.bass_utils import run_bass_kernel_spmd

# problem shape (hardcoded per contract)
B, S, DM, H, DK = 2, 2048, 1024, 16, 64
N_CORES = 8
GROUPS = 4              # head groups (tensor-parallel)
HL = H // GROUPS        # 4 local heads per core
DL = HL * DK            # 256 local concat width
P = 128
SJ = S // P             # 16 strips of 128 along j (keys) and i (out rows)
MT = DM // P            # 8 contraction tiles for projections
DPT = DL // P           # 2 partition tiles of the local concat dim
SCALE = 1.0 / 8.0       # 1/sqrt(DK)

f32 = mybir.dt.float32
f32r = mybir.dt.float32r
bf16 = mybir.dt.bfloat16
AF = mybir.ActivationFunctionType

# Projection stage (inputs + projection weights) in bf16: halves the input
# DMA (the critical-path prefix) at ~2e-3 relative error. Attention and
# output projection stay TF32.
PROJ_BF16 = True
PROJ_DT = bf16 if PROJ_BF16 else f32r

import os as _os
# Reuse PE stationary weights across same-lhsT matmul runs by suppressing
# the per-matmul LDWEIGHTS (InstMatmult.ldweights=False on the trailing
# matmuls of each run).
LDW_REUSE = _os.environ.get("LDW_REUSE", "1") == "1"
# Which j parity routes its softmax rowsums to the DVE (the other parity
# uses the exp's ACT-side accumulator). 2 = all rowsums on ACT.
ROWSUM_DVE_PARITY = int(_os.environ.get("ROWSUM_DVE_PARITY", "1"))

_PROGRAM = None


def _dedupe_ldweights(nc):
    """Drop InstLdweights that reload the exact weights already resident.

    Tile's legalizer splits every matmul into (InstLdweights, InstMatmult
    ldweights=False); each reload costs ~120-180ns serialized into the PE
    stream. When consecutive LDWEIGHTS on the PE stream have identical
    weight APs (the kernel emits same-lhsT matmul runs for scores, attn@V
    and the output projection), the duplicates are pure overhead: the
    array still holds the weights (nothing else writes it), and the WAR
    protection on the SBUF region anchors on the matmuls (Tile tracked
    them as the lhsT readers), so dropping the reload is safe. Waits and
    sem updates of a dropped LDWEIGHTS move onto a NOP in its place."""
    n = 0
    for f in nc.m.functions:
        for blk in f.blocks:
            last_key = None
            new_insts = []
            for inst in blk.instructions:
                if getattr(inst, "engine", None) == mybir.EngineType.PE:
                    tn = type(inst).__name__
                    if tn == "InstLdweights":
                        key = (str(inst.ins[0]), str(inst.tile_position),
                               str(inst.perf_mode), str(inst.is_transpose))
                        if key == last_key:
                            si = inst.sync_info
                            if si is not None and (si.on_wait or si.on_update):
                                new_insts.append(mybir.InstNoOp(
                                    name=f"{inst.name}-ldwskip",
                                    engine=inst.engine,
                                    sync_info=si,
                                    bass_nofuse=True))
                            n += 1
                            continue
                        last_key = key
                    elif tn == "InstMatmult":
                        if inst.is_transpose:
                            last_key = None
                    elif tn in ("InstNoOp", "InstEventSemaphore"):
                        pass
                    else:
                        last_key = None
                new_insts.append(inst)
            blk.instructions[:] = new_insts
    return n


def _split_excess_waits(nc, max_waits=1):
    """walrus in this container rejects >1 semaphore wait per instruction
    (e.g. the Tile kernel-tail Drain); move extras onto same-engine NOPs."""
    n_split = 0
    for f in nc.m.functions:
        for blk in f.blocks:
            new_insts = []
            for inst in blk.instructions:
                si = getattr(inst, "sync_info", None)
                if si is not None and si.on_wait and len(si.on_wait) > max_waits:
                    waits = list(si.on_wait)
                    extra, keep = waits[:-max_waits], waits[-max_waits:]
                    for i in range(0, len(extra), max_waits):
                        chunk = extra[i:i + max_waits]
                        nop = mybir.InstNoOp(
                            name=f"{inst.name}-ws{n_split}-{i}",
                            engine=inst.engine,
                            sync_info=mybir.SyncInfo(on_wait=chunk, on_update=[]),
                            bass_nofuse=True,
                        )
                        new_insts.append(nop)
                    si.on_wait = keep
                    n_split += 1
                new_insts.append(inst)
            blk.instructions[:] = new_insts
    return n_split


def emit_iter(nc, tc, it, const, sb, stat, outp, inp, vtp, attp, pps, ppx,
              qT_in, kT_in, vT_in, wo_d, O_d,
              wq_sb, wk_sb, load_wv, bq_sb, bk_sb, bv_sb, w_slice, wo_cell):
    """One full attention iteration (tile names suffixed _r{it} so the
    program body can be repeated for steady-state timing; tags are shared
    so pool buffers rotate/serialize across reps)."""
    R = f"_r{it}"

    # ---------------- persistent activations ----------------
    # qT split by i-half, kT by i-quarter (j-group): finer tiles give
    # the scheduler finer dependencies, so scores start before the
    # whole projection finishes.
    # bufs=2: the next rep's projections may write the other buffer while
    # this rep's scores still read this one (removes the cross-rep WAR).
    qT_sb = [[sb.tile([P, 512], bf16, name=f"qT{dp}_{i4}{R}",
                      tag=f"qT{dp}_{i4}", bufs=2) for i4 in range(4)]
             for dp in range(DPT)]
    kT_sb = [[sb.tile([P, 512], bf16, name=f"kT{dp}_{jg}{R}",
                      tag=f"kT{dp}_{jg}", bufs=2) for jg in range(4)]
             for dp in range(DPT)]
    # v packed per j-group of 4: v4_sb[jg][p, jj*DL + d'] holds
    # v[jg*512 + jj*128 + p, d']
    v4_sb = [sb.tile([P, 4 * DL], bf16, name=f"v{jg}{R}", tag=f"v{jg}")
             for jg in range(4)]
    xT_sb = [[sb.tile([P, 1024], bf16, name=f"xT{hp}_{ih}{R}",
                      tag=f"xT{hp}_{ih}") for ih in range(2)]
             for hp in range(DPT)]

    # ---------------- projections ----------------
    # dst[d', i] = W.T @ inT ; contraction over m on partitions.
    vT_sb = [vtp.tile([P, S], bf16, name=f"vT{dp}{R}", tag=f"vT{dp}")
             for dp in range(DPT)]

    ENGQ = {"q": nc.sync, "k": nc.sync, "v": nc.sync}

    def load_in_chunk(win, nm, i4):
        # one DMA: all 8 m-blocks of columns [i0, i0+512)
        t = inp.tile([P, MT * 512], PROJ_DT, name=f"{nm}in{i4}{R}",
                     tag="pin")
        src = win.ap().rearrange("(t p) c -> p t c", p=P)
        ENGQ[nm].dma_start(
            t[:].rearrange("p (t c) -> p t c", t=MT),
            src[:, :, i4 * 512:(i4 + 1) * 512])
        return t

    qk_prio = tc.high_priority()
    qk_prio.__enter__()
    QK_ORDER = [("q", 0), ("k", 0), ("q", 1), ("q", 2), ("q", 3),
                ("k", 1), ("k", 2), ("k", 3)]
    for nm, i4 in QK_ORDER:
        if True:
            win, w_sb, b_sb = ((qT_in, wq_sb, bq_sb) if nm == "q"
                               else (kT_in, wk_sb, bk_sb))
            ch = load_in_chunk(win, nm, i4)
            for dp in range(DPT):
                ps = ppx.tile([P, 512], f32, name=f"ps{nm}{i4}_{dp}{R}",
                              tag="px", bufs=2)
                for m in range(MT):
                    nc.tensor.matmul(
                        ps[:], w_slice(w_sb, m, dp),
                        ch[:, m * 512:(m + 1) * 512],
                        start=(m == 0), stop=(m == MT - 1))
                dst = (qT_sb if nm == "q" else kT_sb)[dp][i4][:]
                nc.vector.tensor_scalar_add(dst, ps[:], b_sb[:, dp:dp + 1])

    # vT[d', j] = Wv.T @ valueT (bias folded in, bf16 out), then one
    # SBUF->SBUF bf16 DMA-transpose per (i4, dp) covering 4 j-tiles,
    # dispatched on the ACT HWDGE queue to keep SP free for inputs.
    qk_prio.__exit__(None, None, None)
    wv_sb = load_wv()
    for i4 in range(4):
        i0 = i4 * 512
        ch = load_in_chunk(vT_in, "v", i4)
        for dp in range(DPT):
            ps = ppx.tile([P, 512], f32, name=f"psvt{i4}_{dp}{R}",
                          tag="px", bufs=2)
            for m in range(MT):
                nc.tensor.matmul(
                    ps[:], w_slice(wv_sb, m, dp),
                    ch[:, m * 512:(m + 1) * 512],
                    start=(m == 0), stop=(m == MT - 1))
            nc.vector.tensor_scalar_add(
                vT_sb[dp][:, i0:i0 + 512], ps[:], bv_sb[:, dp:dp + 1])
        for dp in range(DPT):
            out_view = v4_sb[i4][:].rearrange(
                "p (j c) -> p j c", j=4)[:, :,
                                         dp * P:(dp + 1) * P]
            # sync queue: keeps the ACT queue free -- ACT (exp) is the
            # whole-kernel critical path and each queued DMA costs ~1.2us
            # of its engine time.
            nc.sync.dma_start(
                out_view, vT_sb[dp][:, i0:i0 + 512], transpose=True)

    # ---------------- attention ----------------
    # Heads run in sibling pairs (2hp, 2hp+1) whose kT/qT slices live at
    # partitions 0-63 / 64-127 (PE row tiles 0/64). Per (head, j) the four
    # scores matmuls (2 i-halves x 2 i-chunks) share one kT stationary
    # slice and the four attn@V matmuls share one vsc slice: with
    # LDW_REUSE the trailing matmuls set InstMatmult.ldweights=False so
    # walrus skips the per-matmul LDWEIGHTS reload (HW trace showed the
    # 768 reloads/rep serialize ~130ns each into the PE stream).
    # Softmax rowsums ride the exp's ACT-side accumulator on even j and a
    # DVE tensor_scalar on odd j, balancing the two near-critical engines.
    # attn@V accumulates both i-halves into two [128, 1024] psums held for
    # the whole pair (sibling heads at psum partitions 0-63/64-127).

    MULT = mybir.AluOpType.mult
    ADD = mybir.AluOpType.add

    def head_scores(hp, hh, j):
        """scores + exp + rowsum for one head, full i range (2 psum tiles)."""
        jg, jr = divmod(j, 4)
        h = hp * 2 + hh
        base = hh * 64
        lhs = kT_sb[hp][jg][base:base + 64, jr * P:(jr + 1) * P]
        pss = []
        first = True
        for ih in range(2):
            ps = pps.tile([P, 1024], f32, name=f"ps{h}_{j}_{ih}{R}",
                          tag="ps")
            for i5 in range(2):
                mm = nc.tensor.matmul(
                    ps[:, i5 * 512:(i5 + 1) * 512], lhs,
                    qT_sb[hp][ih * 2 + i5][base:base + 64, :],
                    start=True, stop=True)
                if LDW_REUSE and not first:
                    mm.ins.ldweights = False
                first = False
            pss.append(ps)
        outs = []
        for ih, ps in enumerate(pss):
            a = attp.tile([P, 1024], bf16, name=f"att{h}_{j}_{ih}{R}",
                          tag=f"att{ih}", bufs=(12 if ih == 0 else 33))
            rsh = stat.tile([P, 1], f32, name=f"rsh{h}_{j}_{ih}{R}",
                            tag="rsh", bufs=16)
            if j % 2 == ROWSUM_DVE_PARITY:
                nc.scalar.activation(a[:], ps[:], AF.Exp, scale=SCALE)
                nc.vector.tensor_scalar(a[:], a[:], 1.0, 0.0, MULT, ADD,
                                        accum_out=rsh[:])
            else:
                nc.scalar.activation(a[:], ps[:], AF.Exp, scale=SCALE,
                                     accum_out=rsh[:])
            outs.append((a, rsh))
        return outs

    def head_finish_a(hp, hh, j, xph, outs):
        """softmax divisor onto v, then attn@V for i-half 0; saves the
        i-half-1 eT and the vsc tile for phase B."""
        h = hp * 2 + hh
        jg, jr = divmod(j, 4)
        rs = stat.tile([P, 1], f32, name=f"rs{h}_{j}{R}", tag="rs")
        nc.vector.tensor_add(rs[:], outs[0][1][:], outs[1][1][:])
        rc = stat.tile([P, 1], f32, name=f"rc{h}_{j}{R}", tag="rc")
        nc.vector.reciprocal(rc[:], rs[:])
        vsc = attp.tile([P, 64], bf16, name=f"vsc{h}_{j}{R}", tag="vsc",
                        bufs=34)
        nc.vector.tensor_scalar_mul(
            vsc[:],
            v4_sb[jg][:, jr * DL + h * 64:jr * DL + (h + 1) * 64],
            rc[:])
        vsc_t[h][j] = vsc
        a1_t[h][j] = outs[1][0]
        for i5 in range(2):
            nc.tensor.matmul(
                xph[hh * 64:(hh + 1) * 64, i5 * 512:(i5 + 1) * 512],
                vsc[:], outs[0][0][:, i5 * 512:(i5 + 1) * 512],
                start=(j == 0), stop=(j == SJ - 1),
                skip_group_check=True)

    def head_finish_b(hp, hh, j, xph):
        """attn@V for i-half 1 from the saved eT/vsc (pure PE work)."""
        h = hp * 2 + hh
        a = a1_t[h][j]
        for i5 in range(2):
            nc.tensor.matmul(
                xph[hh * 64:(hh + 1) * 64, i5 * 512:(i5 + 1) * 512],
                vsc_t[h][j][:], a[:, i5 * 512:(i5 + 1) * 512],
                start=(j == 0), stop=(j == SJ - 1),
                skip_group_check=True)

    vsc_t = [[None] * SJ for _ in range(HL)]
    a1_t = [[None] * SJ for _ in range(HL)]

    # ---------------- output projection constants ----------------
    # bo is added on the host during unshard (a K=1 ones-row matmul for it
    # here would cost 16384 PE rows ~ 7us).
    if not wo_cell:
        wo_sb = const.tile([P, DPT * DM], bf16, name="wo", tag="wo")
        nc.sync.dma_start(
            wo_sb[:].rearrange("p (t c) -> p t c", t=DPT),
            wo_d.ap().rearrange("(t p) c -> p t c", p=P))
        wo_cell.append(wo_sb)
    wo_sb = wo_cell[0]

    def emit_outproj(jts):
        for jt in jts:
            ot = outp.tile([P, DM], f32, name=f"ot{jt}{R}", tag="ot")
            ps = ppx.tile([P, DM], f32, name=f"pso{jt}{R}", tag="xps",
                          bufs=1)
            jh, jo = divmod(jt, 8)
            for cpt in range(DPT):
                first = True
                for n5 in range(2):
                    no = n5 * 512
                    mm = nc.tensor.matmul(
                        ps[:, no:no + 512],
                        xT_sb[cpt][jh][:, jo * P:(jo + 1) * P],
                        wo_sb[:, cpt * DM + no:cpt * DM + no + 512],
                        start=(cpt == 0), stop=(cpt == DPT - 1))
                    if LDW_REUSE and not first:
                        mm.ins.ldweights = False
                    first = False
            nc.vector.tensor_copy(ot[:], ps[:])
            oq = nc.sync if jt % 2 == 0 else nc.gpsimd
            oq.dma_start(O_d.ap()[jt * P:(jt + 1) * P, :], ot[:])

    def alloc_xps(hp, ih):
        return ppx.tile([P, 1024], f32, name=f"xp{ih}_{hp}{R}", tag="xps",
                        bufs=1)

    def copy_xps(hp, ih, xph):
        nc.vector.tensor_copy(xT_sb[hp][ih][:], xph[:])

    prio = tc.high_priority()
    prio.__enter__()

    # Per pair: phase A runs scores + ALL exps (both i-halves; the rowsum
    # needs the full i range) plus attn@V for i-half 0; phase B is the
    # pure-PE attn@V for i-half 1 from saved eT/vsc tiles. The single-slot
    # "xps" psum rotation (A -> B -> next pair) leaves 2 banks for the
    # next rep's projections to overlap this rep's attention.
    for hp in range(2):
        xph = alloc_xps(hp, 0)
        for j in range(SJ):
            oA = head_scores(hp, 0, j)
            oB = head_scores(hp, 1, j)
            head_finish_a(hp, 0, j, xph, oA)
            head_finish_a(hp, 1, j, xph, oB)
        copy_xps(hp, 0, xph)
        xph = alloc_xps(hp, 1)
        for j in range(SJ):
            head_finish_b(hp, 0, j, xph)
            head_finish_b(hp, 1, j, xph)
        copy_xps(hp, 1, xph)
    prio.__exit__(None, None, None)

    # output projection trails the whole rep: pure PE+DVE+DMA work that
    # overlaps the next rep's projections (which own the high-prio lane).
    emit_outproj(range(SJ))


def build_program(split_waits=True, reps=1):
    nc = bass.Bass("TRN2", target_bir_lowering=False, debug=False)

    qT_in = nc.dram_tensor("qT_in", [DM, S], PROJ_DT, kind="ExternalInput")
    kT_in = nc.dram_tensor("kT_in", [DM, S], PROJ_DT, kind="ExternalInput")
    vT_in = nc.dram_tensor("vT_in", [DM, S], PROJ_DT, kind="ExternalInput")
    wq_d = nc.dram_tensor("wq", [DM, DL], PROJ_DT, kind="ExternalInput")
    wk_d = nc.dram_tensor("wk", [DM, DL], PROJ_DT, kind="ExternalInput")
    wv_d = nc.dram_tensor("wv", [DM, DL], PROJ_DT, kind="ExternalInput")
    wo_d = nc.dram_tensor("wo", [DL, DM], bf16, kind="ExternalInput")
    bq_d = nc.dram_tensor("bq", [DL, 1], f32, kind="ExternalInput")
    bk_d = nc.dram_tensor("bk", [DL, 1], f32, kind="ExternalInput")
    bv_d = nc.dram_tensor("bv", [DL, 1], f32, kind="ExternalInput")
    O_d = nc.dram_tensor("O", [S, DM], f32, kind="ExternalOutput")

    with tile.TileContext(nc) as tc:
        with (
            tc.tile_pool(name="const", bufs=1) as const,
            tc.tile_pool(name="persist", bufs=1) as sb,
            tc.tile_pool(name="stat", bufs=6) as stat,
            tc.tile_pool(name="outp", bufs=3) as outp,
            tc.tile_pool(name="inp", bufs=2) as inp,
            tc.tile_pool(name="vtp", bufs=1) as vtp,
            tc.tile_pool(name="attp", bufs=20) as attp,
            tc.tile_pool(name="pps", bufs=2, space="PSUM") as pps,
            tc.tile_pool(name="ppx", bufs=1, space="PSUM") as ppx,
        ):
            # ---------------- constants ----------------
            # One DMA per weight: DRAM [(t p), c] -> SBUF [p, (t c)] so the
            # m-th 128-row block lands at free offset m*DL.
            def load_w(dram, nm, dt_, cols, eng):
                t = const.tile([P, MT * cols], dt_, name=nm, tag=nm)
                eng.dma_start(
                    t[:].rearrange("p (t c) -> p t c", t=MT),
                    dram.ap().rearrange("(t p) c -> p t c", p=P))
                return t

            wq_sb = load_w(wq_d, "wq", PROJ_DT, DL, nc.scalar)  # [128, 8*256]
            wk_sb = load_w(wk_d, "wk", PROJ_DT, DL, nc.scalar)
            bq_sb = const.tile([P, DPT], f32, name="bq", tag="bq")
            nc.sync.dma_start(
                bq_sb[:].rearrange("p (t c) -> p t c", t=DPT),
                bq_d.ap().rearrange("(t p) c -> p t c", p=P))
            bk_sb = const.tile([P, DPT], f32, name="bk", tag="bk")
            nc.sync.dma_start(
                bk_sb[:].rearrange("p (t c) -> p t c", t=DPT),
                bk_d.ap().rearrange("(t p) c -> p t c", p=P))
            bv_sb = const.tile([P, DPT], f32, name="bv", tag="bv")
            nc.sync.dma_start(
                bv_sb[:].rearrange("p (t c) -> p t c", t=DPT),
                bv_d.ap().rearrange("(t p) c -> p t c", p=P))
            wv_cell = []

            def load_wv():
                if not wv_cell:
                    wv_cell.append(load_w(wv_d, "wv", PROJ_DT, DL, nc.sync))
                return wv_cell[0]

            def w_slice(w, m, dp):
                return w[:, m * DL + dp * P:m * DL + (dp + 1) * P]

            wo_cell = []

            for it in range(reps):
                emit_iter(nc, tc, it, const, sb, stat, outp, inp, vtp, attp,
                          pps, ppx, qT_in, kT_in, vT_in, wo_d, O_d,
                          wq_sb, wk_sb, load_wv, bq_sb, bk_sb, bv_sb, w_slice,
                          wo_cell)

    if LDW_REUSE:
        _dedupe_ldweights(nc)
    if split_waits:
        _split_excess_waits(nc)
    return nc


def _get_program():
    global _PROGRAM
    if _PROGRAM is None:
        _PROGRAM = build_program()
    return _PROGRAM


def _tf32(x):
    """Round fp32 -> TF32 (10-bit mantissa), round-to-nearest-even."""
    x = np.ascontiguousarray(np.asarray(x, dtype=np.float32))
    u = x.view(np.uint32)
    r = ((u >> 13) & 1).astype(np.uint32)
    u2 = ((u + np.uint32(0x0FFF) + r) & np.uint32(0xFFFFE000))
    return u2.view(np.float32)


def shard_inputs(inputs):
    """FULL inputs -> per-core in_maps (list of 8 dicts)."""
    q = np.asarray(inputs["query"], dtype=np.float32)
    k = np.asarray(inputs["key"], dtype=np.float32)
    v = np.asarray(inputs["value"], dtype=np.float32)
    Wq = np.asarray(inputs["Wq"], dtype=np.float32)
    Wk = np.asarray(inputs["Wk"], dtype=np.float32)
    Wv = np.asarray(inputs["Wv"], dtype=np.float32)
    Wo = np.asarray(inputs["Wo"], dtype=np.float32)
    bq = np.asarray(inputs["bq"], dtype=np.float32)
    bk = np.asarray(inputs["bk"], dtype=np.float32)
    bv = np.asarray(inputs["bv"], dtype=np.float32)
    bo = np.asarray(inputs["bo"], dtype=np.float32)

    if PROJ_BF16:
        import ml_dtypes

        def _proj_cast(x):
            return np.ascontiguousarray(np.asarray(x, np.float32)).astype(
                ml_dtypes.bfloat16)
    else:
        _proj_cast = _tf32

    qT = [_proj_cast(q[b].T) for b in range(B)]
    kT = [_proj_cast(k[b].T) for b in range(B)]
    vT = [_proj_cast(v[b].T) for b in range(B)]

    in_maps = []
    for c in range(N_CORES):
        b, g = c // GROUPS, c % GROUPS
        sl = slice(g * DL, (g + 1) * DL)
        in_maps.append({
            "qT_in": qT[b],
            "kT_in": kT[b],
            "vT_in": vT[b],
            "wq": _proj_cast(Wq[:, sl]),
            "wk": _proj_cast(Wk[:, sl]),
            "wv": _proj_cast(Wv[:, sl]),
            "wo": _proj_cast(Wo[sl, :]),
            "bq": np.ascontiguousarray(bq[sl].reshape(DL, 1)),
            "bk": np.ascontiguousarray(bk[sl].reshape(DL, 1)),
            "bv": np.ascontiguousarray(bv[sl].reshape(DL, 1)),
        })
    return in_maps


def unshard_output(results, bo):
    """results: list of 8 dicts with 'O' [S, DM] -> full [B, S, DM].
    bo is added here (host) -- cheaper than a K=1 PE matmul on device."""
    out = np.zeros((B, S, DM), np.float32)
    for c in range(N_CORES):
        out[c // GROUPS] += results[c]["O"]
    out += np.asarray(bo, np.float32)
    return out


def kernel(**inputs):
    nc = _get_program()
    in_maps = shard_inputs(inputs)
    res = run_bass_kernel_spmd(nc, in_maps, core_ids=list(range(N_CORES)))
    return unshard_output(res.results, inputs["bo"])

